# revision 1
# baseline (speedup 1.0000x reference)
"""BWGNN_Hetero Trainium2 kernel: 8-core SPMD, node-sharded graph/data parallel.

Key algorithmic reduction: the 5 beta-wavelet filters are linear combinations
of {f0, f1=L f0, f2=L f1} (L = normalized Laplacian), so each relation needs
only 2 sparse hops, not 8.

Host: shards nodes (6250/core, padded 6272), pre-transposes features to
[feature, node] layout, packs all weights into one SBUF image, groups each
core's edges by destination tile, and builds gather-index + one-hot indicator
streams (padded to common per-tile window counts so the single SPMD program
fits all cores).

Device per core: fused LSTMs -> linear head -> per relation: fp16 message
AllGather -> dma_gather by src -> one-hot fp16 indicator matmuls (f32 PSUM)
as the dst segment-sum -> f1, f2 -> 5-filter attention epilogue -> partial
outputs. Host sums partials, adds b_lin6, unshards.
"""
import sys
sys.path.insert(0, '/opt/trn_rl_repo')
sys.path.insert(0, '/root/problem')

import numpy as np

import concourse.bacc as bacc
import concourse.bass as bass
import concourse.mybir as mybir
import concourse.tile as tile
from concourse.bass_utils import run_bass_kernel_spmd

F32 = mybir.dt.float32
F16 = mybir.dt.float16
I16 = mybir.dt.int16
A = mybir.AluOpType
AF = mybir.ActivationFunctionType

NCORES = 8
N, E, R, T = 50000, 800000, 3, 16
IV, IS, IP, H, C = 64, 64, 32, 128, 2
NL = N // NCORES            # 6250
NT = 49                     # dst tiles per core
NLP = NT * 128              # 6272 padded local nodes
NGP = NLP * NCORES          # 50176 padded global rows in AllGather output
LO_LIM = 32768              # int16 gather index limit
GS = 2                      # dst tiles per gather group
CHUNKS = [(i * 512, 512) for i in range(12)] + [(6144, 128)]

CTRUE = [[0.8, -0.5, 0.0],
         [3.0, -3.0, 0.75],
         [0.0, 3.0, -1.5],
         [0.0, 0.0, 0.75],
         [-0.2, 0.5, 0.0]]


def _wrap_idx(idx):
    """[n] int16 -> [128, ceil(n/16)] wrapped (i -> [i%16, i//16]) + replicated x8."""
    n = len(idx)
    L = max(1, (n + 15) // 16)
    a = np.zeros((16, L), np.int16)
    for p in range(16):
        vals = idx[p::16]
        a[p, :len(vals)] = vals
    return np.tile(a, (8, 1))


class WPack:
    def __init__(self):
        self.cols = []
        self.off = 0
        self.slots = {}

    def add(self, name, mat, row0=0):
        mat = np.asarray(mat, np.float32)
        k, m = mat.shape
        assert row0 + k <= 128
        buf = np.zeros((128, m), np.float32)
        buf[row0:row0 + k] = mat
        self.cols.append(buf)
        self.slots[name] = (row0, k, self.off, m)
        self.off += m

    def image(self):
        return np.concatenate(self.cols, axis=1)


def _prep(inp):
    g = {k: np.asarray(v) for k, v in inp.items()}
    wp = WPack()

    groff = {0: 0, 1: 64, 2: 192, 3: 128}   # our gate order [i, f, o, gg] -> torch rows
    lv = np.zeros((128, 256), np.float32)
    ls = np.zeros((128, 256), np.float32)
    for gi in range(4):
        ro = groff[gi]
        lv[0:64, gi * 64:(gi + 1) * 64] = g['Whh_v'][ro:ro + 64, :].T
        lv[64:128, gi * 64:(gi + 1) * 64] = g['Wih_v'][ro:ro + 64, :].T
        ls[0:64, gi * 64:(gi + 1) * 64] = g['Wih_s'][ro:ro + 64, :].T
        ls[64:128, gi * 64:(gi + 1) * 64] = g['Whh_s'][ro:ro + 64, :].T
    wp.add('lstm_v', lv)
    wp.add('lstm_s', ls)
    wp.add('lin', g['W_lin'].T)                       # rows 0:64 (rhs = h_v at base 0)
    wp.add('lin1', g['W_lin1'].T, row0=64)            # rows 64:128 (rhs = h_s at base 64)
    wp.add('pers', g['W_pers'].T)
    wp.add('lin2a', g['W_lin2'][:, 0:128].T)
    wp.add('lin2b', g['W_lin2'][:, 128:256].T)
    wp.add('lin3a', g['W_lin3'][:, 0:128].T)
    wp.add('lin3b', g['W_lin3'][:, 128:256].T)
    wp.add('lin4a', g['W_lin4'][:, 0:128].T)
    wp.add('lin4bd', (g['W_lin4'][:, 128:256] + g['W_lin4'][:, 384:512]).T)
    wp.add('lin4c', g['W_lin4'][:, 256:384].T)
    for r in range(R):
        for o in range(5):
            for j in range(3):
                if CTRUE[o][j] != 0.0:
                    wp.add(f'wf1_{r}_{o}_{j}', (CTRUE[o][j] * g['Wf1'][r]).T)
        wp.add(f'wf2_{r}', g['Wf2'][r][:, None])
        wp.add(f'lin5_{r}', g['W_lin5'][r].T)
    for k in range(6):
        wp.add(f'lin6_{k}', g['W_lin6'][:, k * 128:(k + 1) * 128].T)
    wp.add('ident', np.eye(128, dtype=np.float32))
    wp.add('one11', np.ones((1, 1), np.float32))
    for o in range(5):
        for j in range(3):
            wp.add(f'c_{o}_{j}', np.array([[CTRUE[o][j]]], np.float32))
    wimg = wp.image()

    bcols, blist = {}, []

    def addb(name, vec):
        bcols[name] = len(blist)
        v = np.zeros((128, 1), np.float32)
        v[:len(vec), 0] = np.asarray(vec, np.float32).ravel()
        blist.append(v)

    bv = g['bih_v'] + g['bhh_v']
    bs = g['bih_s'] + g['bhh_s']
    for gi in range(4):
        ro = groff[gi]
        addb(f'bg{gi}', np.concatenate([bv[ro:ro + 64], bs[ro:ro + 64]]))
    addb('b_lin', g['b_lin'])
    addb('b_lin1', g['b_lin1'])
    addb('b_pers', g['b_pers'])
    addb('b2', g['b_lin2'])
    addb('b3', g['b_lin3'])
    addb('b4', g['b_lin4'])
    for r in range(R):
        addb(f'bf1_{r}', g['bf1'][r])
        addb(f'b5_{r}', g['b_lin5'][r])
    bimg = np.concatenate(blist, axis=1)

    src = np.asarray(g['src'], np.int64)
    dst = np.asarray(g['dst'], np.int64)
    gsrc_all = (src // NL) * NLP + (src % NL)

    percore = [dict() for _ in range(NCORES)]
    relmeta = []
    for r in range(R):
        deg = np.bincount(dst[r], minlength=N).astype(np.float32)
        dinv = np.clip(deg, 1.0, None) ** -0.5

        # bucket edges: (core, tile, class)
        per = []
        for c in range(NCORES):
            m = (dst[r] // NL) == c
            sc = gsrc_all[r][m]
            dl = dst[r][m] - c * NL
            tl, col = dl // 128, dl % 128
            tiles = []
            for t in range(NT):
                mt = tl == t
                st_, ct_ = sc[mt], col[mt]
                lo = st_ < LO_LIM
                tiles.append((st_[lo], ct_[lo], st_[~lo] - LO_LIM, ct_[~lo]))
            per.append(tiles)
        # common (max-over-cores) window counts
        lo_w = [max(1, max((len(per[c][t][0]) + 127) // 128 for c in range(NCORES)))
                for t in range(NT)]
        hi_w = [max(1, max((len(per[c][t][2]) + 127) // 128 for c in range(NCORES)))
                for t in range(NT)]
        relmeta.append({'lo_w': lo_w, 'hi_w': hi_w})
        for c in range(NCORES):
            li_s, lc_s, hi_s, hc_s = [], [], [], []
            for t in range(NT):
                li, lc, hi, hc = per[c][t]
                lp = np.zeros(lo_w[t] * 128, np.int64); lp[:len(li)] = li
                lcp = np.full(lo_w[t] * 128, -1, np.int64); lcp[:len(lc)] = lc
                hp = np.zeros(hi_w[t] * 128, np.int64); hp[:len(hi)] = hi
                hcp = np.full(hi_w[t] * 128, -1, np.int64); hcp[:len(hc)] = hc
                li_s.append(lp); lc_s.append(lcp); hi_s.append(hp); hc_s.append(hcp)
            li_s = np.concatenate(li_s); lc_s = np.concatenate(lc_s)
            hi_s = np.concatenate(hi_s); hc_s = np.concatenate(hc_s)

            def mkind(colarr):
                W = len(colarr) // 128
                ind = np.zeros((W * 128, 128), np.float16)
                valid = colarr >= 0
                ind[np.nonzero(valid)[0], colarr[valid]] = 1.0
                return ind
            pc = percore[c]
            pc[f'gidx_lo_{r}'] = _wrap_idx(li_s.astype(np.int16))
            pc[f'gidx_hi_{r}'] = _wrap_idx(hi_s.astype(np.int16))
            pc[f'ind_lo_{r}'] = mkind(lc_s)
            pc[f'ind_hi_{r}'] = mkind(hc_s)
            dp = pc.setdefault('_dinv', np.zeros((128, 2 * R * NT), np.float32))
            dvl = np.ones(NLP, np.float32)
            dvl[:NL] = dinv[c * NL:(c + 1) * NL]
            dp[:, r * NT:(r + 1) * NT] = dvl.reshape(NT, 128).T
            dp[:, R * NT + r * NT:R * NT + (r + 1) * NT] = -dvl.reshape(NT, 128).T

    voc = np.asarray(g['voc_features'], np.float32)
    sms = np.asarray(g['sms_features'], np.float32)
    pers = np.asarray(g['personal_feature'], np.float32)
    cores = []
    for c in range(NCORES):
        pc = percore[c]
        sl = slice(c * NL, (c + 1) * NL)
        vt = np.zeros((T, IV, NLP), np.float32)
        st_ = np.zeros((T, IS, NLP), np.float32)
        vt[:, :, :NL] = voc[sl].transpose(1, 2, 0)
        st_[:, :, :NL] = sms[sl].transpose(1, 2, 0)
        pt = np.zeros((IP, NLP), np.float32)
        pt[:, :NL] = pers[sl].T
        pc['voc'] = vt
        pc['sms'] = st_
        pc['pers'] = pt
        pc['wpack'] = wimg
        pc['bpack'] = bimg
        pc['dpack'] = pc.pop('_dinv')
        cores.append(pc)
    meta = {
        'wp': wp.slots, 'bcols': bcols, 'rel': relmeta,
        'shapes': {k: v.shape for k, v in cores[0].items()},
        'dtypes': {k: v.dtype for k, v in cores[0].items()},
    }
    return meta, cores


def _build(nc, meta):
    sh, dt = meta['shapes'], meta['dtypes']
    WP, BC = meta['wp'], meta['bcols']
    inputs = {k: nc.dram_tensor(k, list(sh[k]), mybir.dt.from_np(np.dtype(dt[k])),
                                kind="ExternalInput") for k in sh}
    out_parts = nc.dram_tensor("out_parts", [4, 2, NLP], F32, kind="ExternalOutput")

    def wsl(wt, name):
        r0, k, off, m = WP[name]
        return wt[r0:r0 + k, off:off + m]

    with tile.TileContext(nc) as tc:
        with (
            tc.tile_pool(name="const", bufs=1) as cpool,
            tc.tile_pool(name="persist", bufs=1) as spool,
            tc.tile_pool(name="dram", bufs=2, space="DRAM") as dpool,
        ):
            wt = cpool.tile([128, sh['wpack'][1]], F32)
            nc.sync.dma_start(wt[:], inputs['wpack'][:])
            bt = cpool.tile([128, sh['bpack'][1]], F32)
            nc.sync.dma_start(bt[:], inputs['bpack'][:])
            dpt = cpool.tile([128, 2 * R * NT], F32)
            nc.sync.dma_start(dpt[:], inputs['dpack'][:])
            onesf16 = cpool.tile([1, 128], F16)
            nc.vector.tensor_copy(onesf16[:], wsl(wt, 'ident')[0:1, :])  # row of identity: [1,0,0..] NO
            # build a true fp16 ones row: copy from f32 ones = c_0_0? use memset
            nc.vector.memset(onesf16[:], 1.0)

            def bias(name):
                return bt[:, BC[name]:BC[name] + 1]

            XR = spool.tile([128, NLP], F32)
            xin_dram = [dpool.tile([128, NLP], F32, tag=f"xin{r}", name=f"xind{r}") for r in range(2)]
            gam_dram = dpool.tile([3, NLP], F16, tag="gam")

            # =============== Phase A ===============
            with (tc.tile_pool(name="stA", bufs=1) as stA,
                  tc.tile_pool(name="wpA", bufs=2) as wpA):
                XHv = stA.tile([128, NLP], F32)
                XHs = stA.tile([128, NLP], F32)
                Cst = stA.tile([128, NLP], F32)
                nc.vector.memset(XHv[0:64, :], 0.0)
                nc.vector.memset(XHs[64:128, :], 0.0)
                nc.vector.memset(Cst[:], 0.0)
                with tc.tile_pool(name="psA", bufs=2, space="PSUM") as psA:
                    for t in range(T):
                        nc.sync.dma_start(XHv[64:128, :], inputs['voc'][t])
                        nc.sync.dma_start(XHs[0:64, :], inputs['sms'][t])
                        for (c0, cw) in CHUNKS:
                            P = [psA.tile([128, 512], F32, tag=f"g{gi}", name=f"Pg{gi}") for gi in range(4)]
                            for gi in range(4):
                                nc.tensor.matmul(P[gi][0:64, :cw],
                                                 lhsT=wsl(wt, 'lstm_v')[:, gi * 64:(gi + 1) * 64],
                                                 rhs=XHv[:, c0:c0 + cw], start=True, stop=True)
                                nc.tensor.matmul(P[gi][64:128, :cw],
                                                 lhsT=wsl(wt, 'lstm_s')[:, gi * 64:(gi + 1) * 64],
                                                 rhs=XHs[:, c0:c0 + cw], start=True, stop=True)
                            TI = wpA.tile([128, 512], F32, tag="TI")
                            TF = wpA.tile([128, 512], F32, tag="TF")
                            TO = wpA.tile([128, 512], F32, tag="TO")
                            TG = wpA.tile([128, 512], F32, tag="TG")
                            nc.scalar.activation(TI[:, :cw], P[0][:, :cw], AF.Sigmoid, bias=bias('bg0'))
                            nc.scalar.activation(TF[:, :cw], P[1][:, :cw], AF.Sigmoid, bias=bias('bg1'))
                            nc.scalar.activation(TO[:, :cw], P[2][:, :cw], AF.Sigmoid, bias=bias('bg2'))
                            nc.scalar.activation(TG[:, :cw], P[3][:, :cw], AF.Tanh, bias=bias('bg3'))
                            u = wpA.tile([128, 512], F32, tag="u")
                            v = wpA.tile([128, 512], F32, tag="v")
                            nc.vector.scalar_tensor_tensor(u[:, :cw], TF[:, :cw], 0.0,
                                                           Cst[:, c0:c0 + cw], op0=A.bypass, op1=A.mult)
                            nc.vector.scalar_tensor_tensor(v[:, :cw], TI[:, :cw], 0.0,
                                                           TG[:, :cw], op0=A.bypass, op1=A.mult)
                            nc.vector.scalar_tensor_tensor(Cst[:, c0:c0 + cw], u[:, :cw], 0.0,
                                                           v[:, :cw], op0=A.bypass, op1=A.add)
                            tcn = wpA.tile([128, 512], F32, tag="tc")
                            nc.scalar.activation(tcn[:, :cw], Cst[:, c0:c0 + cw], AF.Tanh)
                            nc.vector.scalar_tensor_tensor(XHv[0:64, c0:c0 + cw], TO[0:64, :cw], 0.0,
                                                           tcn[0:64, :cw], op0=A.bypass, op1=A.mult)
                            nc.vector.scalar_tensor_tensor(XHs[64:128, c0:c0 + cw], TO[64:128, :cw], 0.0,
                                                           tcn[64:128, :cw], op0=A.bypass, op1=A.mult)

                # ---- Phase A epilogue (inside stA scope: uses XHv/XHs)
                PT = stA.tile([32, NLP], F32)
                nc.sync.dma_start(PT[:], inputs['pers'][:])
                with (tc.tile_pool(name="psB", bufs=2, space="PSUM") as psB,
                      tc.tile_pool(name="psBs", bufs=2, space="PSUM") as psBs):
                    for (c0, cw) in CHUNKS:
                        pxa = psB.tile([128, 512], F32, tag="pa")
                        nc.tensor.matmul(pxa[:, :cw], lhsT=wsl(wt, 'lin'),
                                         rhs=XHv[0:64, c0:c0 + cw], start=True, stop=True)
                        pxp = psB.tile([128, 512], F32, tag="pb")
                        nc.tensor.matmul(pxp[:, :cw], lhsT=wsl(wt, 'pers'),
                                         rhs=PT[:, c0:c0 + cw], start=True, stop=True)
                        pxs = psB.tile([128, 512], F32, tag="pc")
                        nc.tensor.matmul(pxs[:, :cw], lhsT=wsl(wt, 'lin1'),
                                         rhs=XHs[64:128, c0:c0 + cw], start=True, stop=True)
                        XA = wpA.tile([128, 512], F32, tag="XA")
                        XP = wpA.tile([128, 512], F32, tag="XP")
                        XS = wpA.tile([128, 512], F32, tag="XS")
                        nc.scalar.activation(XA[:, :cw], pxa[:, :cw], AF.Lrelu, bias=bias('b_lin'), alpha=0.01)
                        nc.scalar.activation(XP[:, :cw], pxp[:, :cw], AF.Lrelu, bias=bias('b_pers'), alpha=0.01)
                        nc.scalar.activation(XS[:, :cw], pxs[:, :cw], AF.Lrelu, bias=bias('b_lin1'), alpha=0.01)
                        p0 = psB.tile([128, 512], F32, tag="pa")
                        nc.tensor.matmul(p0[:, :cw], lhsT=wsl(wt, 'lin2a'), rhs=XA[:, :cw], start=True, stop=False)
                        nc.tensor.matmul(p0[:, :cw], lhsT=wsl(wt, 'lin2b'), rhs=XP[:, :cw], start=False, stop=True)
                        p1 = psB.tile([128, 512], F32, tag="pb")
                        nc.tensor.matmul(p1[:, :cw], lhsT=wsl(wt, 'lin3a'), rhs=XS[:, :cw], start=True, stop=False)
                        nc.tensor.matmul(p1[:, :cw], lhsT=wsl(wt, 'lin3b'), rhs=XP[:, :cw], start=False, stop=True)
                        p2 = psB.tile([128, 512], F32, tag="pc")
                        nc.tensor.matmul(p2[:, :cw], lhsT=wsl(wt, 'lin4a'), rhs=XA[:, :cw], start=True, stop=False)
                        nc.tensor.matmul(p2[:, :cw], lhsT=wsl(wt, 'lin4bd'), rhs=XP[:, :cw], start=False, stop=False)
                        nc.tensor.matmul(p2[:, :cw], lhsT=wsl(wt, 'lin4c'), rhs=XS[:, :cw], start=False, stop=True)
                        X1c = wpA.tile([128, 512], F32, tag="X1c")
                        X2c = wpA.tile([128, 512], F32, tag="X2c")
                        nc.scalar.activation(XR[:, c0:c0 + cw], p0[:, :cw], AF.Lrelu, bias=bias('b2'), alpha=0.01)
                        nc.scalar.activation(X1c[:, :cw], p1[:, :cw], AF.Lrelu, bias=bias('b3'), alpha=0.01)
                        nc.scalar.activation(X2c[:, :cw], p2[:, :cw], AF.Lrelu, bias=bias('b4'), alpha=0.01)
                        nc.sync.dma_start(xin_dram[0][:, c0:c0 + cw], X1c[:, :cw])
                        nc.sync.dma_start(xin_dram[1][:, c0:c0 + cw], X2c[:, :cw])
                        p6 = psBs.tile([2, 512], F32, tag="p6")
                        nc.tensor.matmul(p6[:, :cw], lhsT=wsl(wt, 'lin6_3'), rhs=XR[:, c0:c0 + cw],
                                         start=True, stop=False)
                        nc.tensor.matmul(p6[:, :cw], lhsT=wsl(wt, 'lin6_4'), rhs=X1c[:, :cw],
                                         start=False, stop=False)
                        nc.tensor.matmul(p6[:, :cw], lhsT=wsl(wt, 'lin6_5'), rhs=X2c[:, :cw],
                                         start=False, stop=True)
                        o6 = wpA.tile([2, 512], F32, tag="o6")
                        nc.scalar.copy(o6[:, :cw], p6[:, :cw])
                        nc.sync.dma_start(out_parts[3, :, c0:c0 + cw], o6[:, :cw])

            # =============== Phase B ===============
            import os as _os
            _PART = _os.environ.get("KERNEL_PART", "ALL")
            with tc.tile_pool(name="stB", bufs=1) as stB:
                fprev = stB.tile([128, NLP], F32)
                fnext = stB.tile([128, NLP], F32)
                for r in range(R if _PART == "ALL" else (1 if _PART in ("B1","B1H1","B1NOG","B1G1") else 0)):
                    relm = meta['rel'][r]
                    lo_w, hi_w = relm['lo_w'], relm['hi_w']
                    lo_off, hi_off = [0], [0]
                    for t in range(NT):
                        lo_off.append(lo_off[-1] + lo_w[t])
                        hi_off.append(hi_off[-1] + hi_w[t])
                    maxlo = max(sum(lo_w[t0:t0 + GS]) for t0 in range(0, NT, GS))
                    maxhi = max(sum(hi_w[t0:t0 + GS]) for t0 in range(0, NT, GS))

                    def dv(t):
                        return dpt[:, r * NT + t:r * NT + t + 1]

                    def ndv(t):
                        return dpt[:, R * NT + r * NT + t:R * NT + r * NT + t + 1]

                    if r > 0:
                        nc.sync.dma_start(XR[:], xin_dram[r - 1][:])

                    for hop in range(2 if _PART in ("ALL","B1") else 1):
                        ml = dpool.tile([NLP, H], F16, tag="mloc")
                        mf = dpool.tile([NGP, H], F16, tag="mfull", addr_space="Shared")
                        fsrc = fprev  # node-major f of previous hop (hop0: filled below)
                        fdst = fprev if hop == 0 else fnext
                        with (tc.tile_pool(name=f"psT{r}{hop}", bufs=2, space="PSUM") as psT,
                              tc.tile_pool(name=f"psG{r}{hop}", bufs=2, space="PSUM") as psG,
                              tc.tile_pool(name=f"wpH{r}{hop}", bufs=2) as wpH):
                            if hop == 0:
                                for t in range(NT):
                                    tr = psT.tile([128, 128], F32, tag="tr")
                                    nc.tensor.transpose(tr[:], XR[:, t * 128:(t + 1) * 128],
                                                        wsl(wt, 'ident'))
                                    nc.vector.tensor_copy(fprev[:, t * 128:(t + 1) * 128], tr[:])
                                    m1 = wpH.tile([128, 128], F16, tag="m1")
                                    nc.vector.tensor_scalar_mul(m1[:], tr[:], dv(t))
                                    nc.sync.dma_start(ml[t * 128:(t + 1) * 128, :], m1[:])
                            else:
                                for t in range(NT):
                                    m1 = wpH.tile([128, 128], F16, tag="m1")
                                    nc.vector.tensor_scalar_mul(m1[:], fprev[:, t * 128:(t + 1) * 128], dv(t))
                                    nc.sync.dma_start(ml[t * 128:(t + 1) * 128, :], m1[:])
                            nc.gpsimd.collective_compute(
                                "AllGather", A.bypass,
                                replica_groups=[list(range(NCORES))],
                                ins=[ml.opt()], outs=[mf.opt()],
                            )
                            for t0g in range(0 if _PART != "B1NOG" else NT, NT if _PART != "B1G1" else GS, GS):
                                tiles = list(range(t0g, min(t0g + GS, NT)))
                                nlo = sum(lo_w[t] for t in tiles)
                                nhi = sum(hi_w[t] for t in tiles)
                                bufs = {}
                                for cls, nwin, mx, woff in (('lo', nlo, maxlo, lo_off[tiles[0]]),
                                                            ('hi', nhi, maxhi, hi_off[tiles[0]])):
                                    it = wpH.tile([128, mx * 8], I16, tag=f"idx{cls}")
                                    nc.sync.dma_start(it[:, :nwin * 8],
                                                      inputs[f'gidx_{cls}_{r}'][:, woff * 8:(woff + nwin) * 8])
                                    gb = wpH.tile([128, mx, 128], F16, tag=f"gb{cls}")
                                    in_ap = mf[0:LO_LIM, :] if cls == 'lo' else mf[LO_LIM:NGP, :]
                                    if _os.environ.get("SKIP_GATHER", "0") != "1":
                                        GW = 8
                                        for w0 in range(0, nwin, GW):
                                            sw = min(GW, nwin - w0)
                                            nc.gpsimd.dma_gather(
                                                out_ap=gb[:, w0:w0 + sw, :], in_ap=in_ap,
                                                idxs_ap=it[:, w0 * 8:(w0 + sw) * 8],
                                                num_idxs=sw * 128, num_idxs_reg=sw * 128,
                                                elem_size=H)
                                    else:
                                        nc.vector.memset(gb[:, :nwin, :], 0.0)
                                    ib = wpH.tile([128, mx, 128], F16, tag=f"ib{cls}")
                                    nc.sync.dma_start(
                                        ib[:, :nwin, :],
                                        inputs[f'ind_{cls}_{r}'].ap()[woff * 128:(woff + nwin) * 128, :]
                                        .rearrange("(w e) d -> e w d", w=nwin))
                                    bufs[cls] = (gb, ib)
                                for t in tiles:
                                    agg = psG.tile([128, 128], F32, tag="agg")
                                    wins = ([('lo', lo_off[t] - lo_off[tiles[0]] + w) for w in range(lo_w[t])]
                                            + [('hi', hi_off[t] - hi_off[tiles[0]] + w) for w in range(hi_w[t])])
                                    for wi, (cls, w) in enumerate(wins):
                                        gb, ib = bufs[cls]
                                        nc.tensor.matmul(agg[:], lhsT=ib[:, w, :], rhs=gb[:, w, :],
                                                         start=(wi == 0), stop=(wi == len(wins) - 1))
                                    nc.vector.scalar_tensor_tensor(
                                        fdst[:, t * 128:(t + 1) * 128], agg[:], ndv(t),
                                        fsrc[:, t * 128:(t + 1) * 128],
                                        op0=A.mult, op1=A.add)

                    if _PART in ("B1H1", "B1NOG", "B1G1"):
                        continue
                    # ---- epilogue pass 1: scores -> gamma (fp16, via DRAM)
                    with (tc.tile_pool(name=f"psEt{r}", bufs=2, space="PSUM") as psEt,
                          tc.tile_pool(name=f"psEw{r}", bufs=1, space="PSUM") as psEw,
                          tc.tile_pool(name=f"psEa{r}", bufs=1, space="PSUM") as psEa,
                          tc.tile_pool(name=f"wpE{r}", bufs=2) as wpE):
                        for (c0, cw) in CHUNKS:
                            nsub = cw // 128
                            F1c = wpE.tile([128, 512], F32, tag="F1c")
                            F2c = wpE.tile([128, 512], F32, tag="F2c")
                            for si in range(nsub):
                                tr = psEt.tile([128, 128], F32, tag="tr")
                                nc.tensor.transpose(tr[:], fprev[:, c0 + si * 128:c0 + (si + 1) * 128],
                                                    wsl(wt, 'ident'))
                                nc.vector.tensor_copy(F1c[:, si * 128:(si + 1) * 128], tr[:])
                                tr2 = psEt.tile([128, 128], F32, tag="tr")
                                nc.tensor.transpose(tr2[:], fnext[:, c0 + si * 128:c0 + (si + 1) * 128],
                                                    wsl(wt, 'ident'))
                                nc.vector.tensor_copy(F2c[:, si * 128:(si + 1) * 128], tr2[:])
                            den_ps = psEa.tile([1, 512], F32, tag="den")
                            g_ps = [psEa.tile([1, 512], F32, tag=f"g{j}", name=f"gps{j}") for j in range(3)]
                            for o in range(5):
                                pso = psEw.tile([128, 512], F32, tag="to")
                                Bsrc = [XR[:, c0:c0 + cw], F1c[:, :cw], F2c[:, :cw]]
                                js = [j for j in range(3) if CTRUE[o][j] != 0.0]
                                for ji, j in enumerate(js):
                                    nc.tensor.matmul(pso[:, :cw], lhsT=wsl(wt, f'wf1_{r}_{o}_{j}'),
                                                     rhs=Bsrc[j], start=(ji == 0), stop=(ji == len(js) - 1))
                                To = wpE.tile([128, 512], F32, tag="To")
                                nc.scalar.activation(To[:, :cw], pso[:, :cw], AF.Tanh, bias=bias(f'bf1_{r}'))
                                psc = psEw.tile([1, 512], F32, tag="sc")
                                nc.tensor.matmul(psc[:, :cw], lhsT=wsl(wt, f'wf2_{r}'), rhs=To[:, :cw],
                                                 start=True, stop=True)
                                eo = wpE.tile([1, 512], F32, tag="eo")
                                nc.scalar.activation(eo[:, :cw], psc[:, :cw], AF.Exp)
                                nc.tensor.matmul(den_ps[:, :cw], lhsT=wsl(wt, 'one11'), rhs=eo[:, :cw],
                                                 start=(o == 0), stop=(o == 4))
                                for j in range(3):
                                    nc.tensor.matmul(g_ps[j][:, :cw], lhsT=wsl(wt, f'c_{o}_{j}'),
                                                     rhs=eo[:, :cw], start=(o == 0), stop=(o == 4))
                            rec = wpE.tile([1, 512], F32, tag="rec")
                            nc.vector.reciprocal(rec[:, :cw], den_ps[:, :cw])
                            for j in range(3):
                                gj = wpE.tile([1, 512], F16, tag="gj")
                                nc.vector.scalar_tensor_tensor(gj[:, :cw], rec[:, :cw], 0.0,
                                                               g_ps[j][:, :cw], op0=A.bypass, op1=A.mult)
                                nc.sync.dma_start(gam_dram[j:j + 1, c0:c0 + cw], gj[:, :cw])

                    # ---- epilogue pass 2
                    with (tc.tile_pool(name=f"psF{r}", bufs=2, space="PSUM") as psF,
                          tc.tile_pool(name=f"wpF{r}", bufs=2) as wpF):
                        for (c0, cw) in CHUNKS:
                            nsub = cw // 128
                            F1c = wpF.tile([128, 512], F32, tag="F1c")
                            F2c = wpF.tile([128, 512], F32, tag="F2c")
                            for si in range(nsub):
                                tr = psF.tile([128, 128], F32, tag="tr")
                                nc.tensor.transpose(tr[:], fprev[:, c0 + si * 128:c0 + (si + 1) * 128],
                                                    wsl(wt, 'ident'))
                                nc.vector.tensor_copy(F1c[:, si * 128:(si + 1) * 128], tr[:])
                                tr2 = psF.tile([128, 128], F32, tag="tr")
                                nc.tensor.transpose(tr2[:], fnext[:, c0 + si * 128:c0 + (si + 1) * 128],
                                                    wsl(wt, 'ident'))
                                nc.vector.tensor_copy(F2c[:, si * 128:(si + 1) * 128], tr2[:])
                            Bsrc = [XR[:, c0:c0 + cw], F1c[:, :cw], F2c[:, :cw]]
                            res = wpF.tile([128, 512], F32, tag="res")
                            tmp = wpF.tile([128, 512], F32, tag="tmp")
                            for j in range(3):
                                gst = wpF.tile([1, 512], F16, tag="gst")
                                nc.sync.dma_start(gst[:, :cw], gam_dram[j:j + 1, c0:c0 + cw])
                                pbj = psF.tile([128, 512], F32, tag="bc")
                                nc.tensor.matmul(pbj[:, :cw], lhsT=onesf16[:], rhs=gst[:, :cw],
                                                 start=True, stop=True)
                                if j == 0:
                                    nc.vector.scalar_tensor_tensor(res[:, :cw], Bsrc[j], 0.0, pbj[:, :cw],
                                                                   op0=A.bypass, op1=A.mult)
                                else:
                                    nc.vector.scalar_tensor_tensor(tmp[:, :cw], Bsrc[j], 0.0, pbj[:, :cw],
                                                                   op0=A.bypass, op1=A.mult)
                                    nc.vector.scalar_tensor_tensor(res[:, :cw], res[:, :cw], 0.0,
                                                                   tmp[:, :cw], op0=A.bypass, op1=A.add)
                            ph = psF.tile([128, 512], F32, tag="hall")
                            nc.tensor.matmul(ph[:, :cw], lhsT=wsl(wt, f'lin5_{r}'), rhs=res[:, :cw],
                                             start=True, stop=True)
                            hall = wpF.tile([128, 512], F32, tag="hall_sb")
                            nc.scalar.activation(hall[:, :cw], ph[:, :cw], AF.Lrelu,
                                                 bias=bias(f'b5_{r}'), alpha=0.01)
                            po = psF.tile([2, 512], F32, tag="po")
                            nc.tensor.matmul(po[:, :cw], lhsT=wsl(wt, f'lin6_{r}'), rhs=hall[:, :cw],
                                             start=True, stop=True)
                            oo = wpF.tile([2, 512], F32, tag="oo")
                            nc.scalar.copy(oo[:, :cw], po[:, :cw])
                            nc.sync.dma_start(out_parts[r, :, c0:c0 + cw], oo[:, :cw])

    nc.compile()


def kernel(**inp):
    meta, cores = _prep(inp)
    nc = bacc.Bacc("TRN2", target_bir_lowering=False, debug=False, num_devices=NCORES)
    _build(nc, meta)
    res = run_bass_kernel_spmd(nc, [dict(c) for c in cores], core_ids=list(range(NCORES)))
    out = np.zeros((N, C), np.float32)
    b6 = np.asarray(inp['b_lin6'], np.float32)
    for c in range(NCORES):
        parts = res.results[c]["out_parts"]
        out[c * NL:(c + 1) * NL] = parts.sum(axis=0).T[:NL] + b6[None, :]
    return out


if __name__ == "__main__":
    # quick self-run against the reference
    import reference
    inputs = {k: np.asarray(v) for k, v in reference.setup_inputs().items()}
    got = kernel(**inputs)
    exp = np.asarray(reference.reference(**inputs))
    err = np.abs(got - exp).max()
    rel = err / max(np.abs(exp).max(), 1e-9)
    print("abs err:", err, "rel err:", rel)



# revision 23
# speedup vs baseline: 1.2128x; 1.2128x over previous
"""BWGNN_Hetero Trainium2 kernel: 8-core SPMD, node-sharded graph/data parallel.

v2: fp16 tensor-engine path (merged M=128 LSTM gate matmuls, fp16 weights/
activations, fp16 feature upload), software-pipelined Phase B (all three
relations' hop-0 AllGathers issued at the end of Phase A; hops interleaved
across relations so collectives and tensor epilogues hide under the gpsimd
dma_gather stream), single fused attention epilogue pass, and one dma_gather
call per (tile-group, class).

Algorithmic reduction: the 5 beta-wavelet filters are linear combinations of
{f0, f1=L f0, f2=L f1} (L = normalized Laplacian), so each relation needs
only 2 sparse hops.
"""
import sys
sys.path.insert(0, '/opt/trn_rl_repo')
sys.path.insert(0, '/root/problem')

import numpy as np

import concourse.bacc as bacc
import concourse.bass as bass
import concourse.mybir as mybir
import concourse.tile as tile
from concourse.bass_utils import run_bass_kernel_spmd

F32 = mybir.dt.float32
F16 = mybir.dt.float16
I16 = mybir.dt.int16
A = mybir.AluOpType
AF = mybir.ActivationFunctionType

NCORES = 8
N, E, R, T = 50000, 800000, 3, 16
IV, IS, IP, H, C = 64, 64, 32, 128, 2
NL = N // NCORES            # 6250
NT = 49                     # dst tiles per core
NLP = NT * 128              # 6272 padded local nodes
NGP = NLP * NCORES          # 50176 padded global rows in AllGather output
LO_LIM = 32768              # int16 gather index limit
GS = 2                      # dst tiles per gather group
CHUNKS = [(i * 512, 512) for i in range(12)] + [(6144, 128)]

CTRUE = [[0.8, -0.5, 0.0],
         [3.0, -3.0, 0.75],
         [0.0, 3.0, -1.5],
         [0.0, 0.0, 0.75],
         [-0.2, 0.5, 0.0]]


def _wrap_idx(idx):
    """[n] int16 -> [128, ceil(n/16)] wrapped (i -> [i%16, i//16]) + replicated x8."""
    n = len(idx)
    L = max(1, (n + 15) // 16)
    a = np.zeros((16, L), np.int16)
    for p in range(16):
        vals = idx[p::16]
        a[p, :len(vals)] = vals
    return np.tile(a, (8, 1))


class WPack:
    def __init__(self, dtype):
        self.dtype = dtype
        self.cols = []
        self.off = 0
        self.slots = {}

    def add(self, name, mat, row0=0):
        mat = np.asarray(mat, self.dtype)
        k, m = mat.shape
        assert row0 + k <= 128
        buf = np.zeros((128, m), self.dtype)
        buf[row0:row0 + k] = mat
        self.cols.append(buf)
        self.slots[name] = (row0, k, self.off, m)
        self.off += m

    def image(self):
        return np.concatenate(self.cols, axis=1)


def _prep(inp):
    g = {k: np.asarray(v) for k, v in inp.items()}
    wp = WPack(np.float16)

    # torch gate-row offsets: i=0, f=64, g=128, o=192
    # XHv rows: [h_v (0:64); x_v (64:128)];  XHs rows: [x_s (0:64); h_s (64:128)]
    def lv_pair(g1, g2):
        m = np.zeros((128, 128), np.float32)
        for ci, ro in enumerate((g1, g2)):
            m[0:64, ci * 64:(ci + 1) * 64] = g['Whh_v'][ro:ro + 64, :].T
            m[64:128, ci * 64:(ci + 1) * 64] = g['Wih_v'][ro:ro + 64, :].T
        return m

    def ls_pair(g1, g2):
        m = np.zeros((128, 128), np.float32)
        for ci, ro in enumerate((g1, g2)):
            m[0:64, ci * 64:(ci + 1) * 64] = g['Wih_s'][ro:ro + 64, :].T
            m[64:128, ci * 64:(ci + 1) * 64] = g['Whh_s'][ro:ro + 64, :].T
        return m

    wp.add('lv_if', lv_pair(0, 64))
    wp.add('lv_og', lv_pair(192, 128))
    wp.add('ls_if', ls_pair(0, 64))
    wp.add('ls_og', ls_pair(192, 128))
    wp.add('lin', g['W_lin'].T)                       # rows 0:64 (rhs = h_v at base 0)
    wp.add('lin1', g['W_lin1'].T, row0=64)            # rows 64:128 (rhs = h_s at base 64)
    wp.add('pers', g['W_pers'].T)
    wp.add('lin2a', g['W_lin2'][:, 0:128].T)
    wp.add('lin2b', g['W_lin2'][:, 128:256].T)
    wp.add('lin3a', g['W_lin3'][:, 0:128].T)
    wp.add('lin3b', g['W_lin3'][:, 128:256].T)
    wp.add('lin4a', g['W_lin4'][:, 0:128].T)
    wp.add('lin4bd', (g['W_lin4'][:, 128:256] + g['W_lin4'][:, 384:512]).T)
    wp.add('lin4c', g['W_lin4'][:, 256:384].T)
    for r in range(R):
        for o in range(5):
            for j in range(3):
                if CTRUE[o][j] != 0.0:
                    wp.add(f'wf1_{r}_{o}_{j}', (CTRUE[o][j] * g['Wf1'][r]).T)
        wp.add(f'wf2_{r}', g['Wf2'][r][:, None])
        wp.add(f'lin5_{r}', g['W_lin5'][r].T)
    for k in range(6):
        wp.add(f'lin6_{k}', g['W_lin6'][:, k * 128:(k + 1) * 128].T)
    wp.add('ident', np.eye(128, dtype=np.float32))
    wp.add('one11', np.ones((1, 1), np.float32))
    for o in range(5):
        for j in range(3):
            wp.add(f'c_{o}_{j}', np.array([[CTRUE[o][j]]], np.float32))
    wimg = wp.image()

    bcols, blist = {}, []

    def addb(name, vec):
        bcols[name] = len(blist)
        v = np.zeros((128, 1), np.float32)
        v[:len(vec), 0] = np.asarray(vec, np.float32).ravel()
        blist.append(v)

    bv = g['bih_v'] + g['bhh_v']
    bs = g['bih_s'] + g['bhh_s']
    addb('b_if_v', np.concatenate([bv[0:64], bv[64:128]]))     # [i_v; f_v]
    addb('b_o_v', bv[192:256])
    addb('b_g_v', bv[128:192])
    addb('b_i_s', bs[0:64])
    addb('b_f_s', bs[64:128])
    addb('b_o_s', bs[192:256])
    addb('b_g_s', bs[128:192])
    addb('b_lin', g['b_lin'])
    addb('b_lin1', g['b_lin1'])
    addb('b_pers', g['b_pers'])
    addb('b2', g['b_lin2'])
    addb('b3', g['b_lin3'])
    addb('b4', g['b_lin4'])
    for r in range(R):
        addb(f'bf1_{r}', g['bf1'][r])
        addb(f'b5_{r}', g['b_lin5'][r])
    bimg = np.concatenate(blist, axis=1)

    src = np.asarray(g['src'], np.int64)
    dst = np.asarray(g['dst'], np.int64)
    gsrc_all = (src // NL) * NLP + (src % NL)

    percore = [dict() for _ in range(NCORES)]
    relmeta = []
    for r in range(R):
        deg = np.bincount(dst[r], minlength=N).astype(np.float32)
        dinv = np.clip(deg, 1.0, None) ** -0.5

        # bucket edges: (core, tile, class)
        per = []
        for c in range(NCORES):
            m = (dst[r] // NL) == c
            sc = gsrc_all[r][m]
            dl = dst[r][m] - c * NL
            tl, col = dl // 128, dl % 128
            tiles = []
            for t in range(NT):
                mt = tl == t
                st_, ct_ = sc[mt], col[mt]
                lo = st_ < LO_LIM
                tiles.append((st_[lo], ct_[lo], st_[~lo] - LO_LIM, ct_[~lo]))
            per.append(tiles)
        # common (max-over-cores) window counts
        lo_w = [max(1, max((len(per[c][t][0]) + 127) // 128 for c in range(NCORES)))
                for t in range(NT)]
        hi_w = [max(1, max((len(per[c][t][2]) + 127) // 128 for c in range(NCORES)))
                for t in range(NT)]
        relmeta.append({'lo_w': lo_w, 'hi_w': hi_w})
        for c in range(NCORES):
            li_s, lc_s, hi_s, hc_s = [], [], [], []
            for t in range(NT):
                li, lc, hi, hc = per[c][t]
                lp = np.zeros(lo_w[t] * 128, np.int64); lp[:len(li)] = li
                lcp = np.full(lo_w[t] * 128, -1, np.int64); lcp[:len(lc)] = lc
                hp = np.zeros(hi_w[t] * 128, np.int64); hp[:len(hi)] = hi
                hcp = np.full(hi_w[t] * 128, -1, np.int64); hcp[:len(hc)] = hc
                li_s.append(lp); lc_s.append(lcp); hi_s.append(hp); hc_s.append(hcp)
            li_s = np.concatenate(li_s); lc_s = np.concatenate(lc_s)
            hi_s = np.concatenate(hi_s); hc_s = np.concatenate(hc_s)

            def mkind(colarr):
                W = len(colarr) // 128
                ind = np.zeros((W * 128, 128), np.float16)
                valid = colarr >= 0
                ind[np.nonzero(valid)[0], colarr[valid]] = 1.0
                return ind
            pc = percore[c]
            pc[f'gidx_lo_{r}'] = _wrap_idx(li_s.astype(np.int16))
            pc[f'gidx_hi_{r}'] = _wrap_idx(hi_s.astype(np.int16))
            pc[f'ind_lo_{r}'] = mkind(lc_s)
            pc[f'ind_hi_{r}'] = mkind(hc_s)
            dp = pc.setdefault('_dinv', np.zeros((128, 2 * R * NT), np.float32))
            dvl = np.ones(NLP, np.float32)
            dvl[:NL] = dinv[c * NL:(c + 1) * NL]
            dp[:, r * NT:(r + 1) * NT] = dvl.reshape(NT, 128).T
            dp[:, R * NT + r * NT:R * NT + (r + 1) * NT] = -dvl.reshape(NT, 128).T

    voc = np.asarray(g['voc_features'], np.float16)
    sms = np.asarray(g['sms_features'], np.float16)
    pers = np.asarray(g['personal_feature'], np.float16)
    cores = []
    for c in range(NCORES):
        pc = percore[c]
        sl = slice(c * NL, (c + 1) * NL)
        vt = np.zeros((T, IV, NLP), np.float16)
        st_ = np.zeros((T, IS, NLP), np.float16)
        vt[:, :, :NL] = voc[sl].transpose(1, 2, 0)
        st_[:, :, :NL] = sms[sl].transpose(1, 2, 0)
        pt = np.zeros((IP, NLP), np.float16)
        pt[:, :NL] = pers[sl].T
        pc['voc'] = vt
        pc['sms'] = st_
        pc['pers'] = pt
        pc['wpack'] = wimg
        pc['bpack'] = bimg
        pc['dpack'] = pc.pop('_dinv')
        cores.append(pc)
    meta = {
        'wp': wp.slots, 'bcols': bcols, 'rel': relmeta,
        'shapes': {k: v.shape for k, v in cores[0].items()},
        'dtypes': {k: v.dtype for k, v in cores[0].items()},
    }
    return meta, cores


def _build(nc, meta):
    sh, dt = meta['shapes'], meta['dtypes']
    WP, BC = meta['wp'], meta['bcols']
    inputs = {k: nc.dram_tensor(k, list(sh[k]), mybir.dt.from_np(np.dtype(dt[k])),
                                kind="ExternalInput") for k in sh}
    out_parts = nc.dram_tensor("out_parts", [4, 2, NLP], F32, kind="ExternalOutput")

    def wsl(wt, name):
        r0, k, off, m = WP[name]
        return wt[r0:r0 + k, off:off + m]

    with tile.TileContext(nc) as tc:
        with (
            tc.tile_pool(name="const", bufs=1) as cpool,
            tc.tile_pool(name="persist", bufs=1) as spool,
            tc.tile_pool(name="dram", bufs=1, space="DRAM") as dpool,
        ):
            wt = cpool.tile([128, sh['wpack'][1]], F16)
            nc.sync.dma_start(wt[:], inputs['wpack'][:])
            bt = cpool.tile([128, sh['bpack'][1]], F32)
            nc.sync.dma_start(bt[:], inputs['bpack'][:])
            dpt = cpool.tile([128, 2 * R * NT], F32)
            nc.sync.dma_start(dpt[:], inputs['dpack'][:])
            onesf16 = cpool.tile([1, 128], F16)
            nc.vector.memset(onesf16[:], 1.0)

            def bias(name, p0=0, pn=128):
                return bt[p0:p0 + pn, BC[name]:BC[name] + 1]

            # persistent fp16 node-major filter states: FA_r (f0, later f2), FB_r (f1)
            FA = [spool.tile([128, NLP], F16, name=f"FA{r}") for r in range(R)]
            FB = [spool.tile([128, NLP], F16, name=f"FB{r}") for r in range(R)]
            xin_dram = [dpool.tile([128, NLP], F16, tag=f"xin{r}", name=f"xind{r}")
                        for r in range(R)]
            ml = [[dpool.tile([NLP, H], F16, tag=f"ml{r}h{h}", name=f"mld{r}h{h}")
                   for h in range(2)] for r in range(R)]
            mf = [[dpool.tile([NGP, H], F16, tag=f"mf{r}h{h}", name=f"mfd{r}h{h}",
                              addr_space="Shared") for h in range(2)] for r in range(R)]

            def dv(r, t):
                return dpt[:, r * NT + t:r * NT + t + 1]

            def ndv(r, t):
                return dpt[:, R * NT + r * NT + t:R * NT + r * NT + t + 1]

            # =============== Phase A: fused LSTMs ===============
            with (tc.tile_pool(name="stA", bufs=1) as stA,
                  tc.tile_pool(name="wpA", bufs=2) as wpA):
                XHv = stA.tile([128, NLP], F16)
                XHs = stA.tile([128, NLP], F16)
                Cst = stA.tile([128, NLP], F32)   # rows 64:128 = c_v, rows 0:64 = c_s
                nc.vector.memset(XHv[0:64, :], 0.0)
                nc.vector.memset(XHs[64:128, :], 0.0)
                nc.vector.memset(Cst[:], 0.0)
                with tc.tile_pool(name="psA", bufs=2, space="PSUM") as psA:
                    for t in range(T):
                        nc.sync.dma_start(XHv[64:128, :], inputs['voc'][t])
                        nc.sync.dma_start(XHs[0:64, :], inputs['sms'][t])
                        for (c0, cw) in CHUNKS:
                            PA = psA.tile([128, 512], F32, tag="pa")  # [i_v; f_v]
                            PB = psA.tile([128, 512], F32, tag="pb")  # [o_v; g_v]
                            PC = psA.tile([128, 512], F32, tag="pc")  # [i_s; f_s]
                            PD = psA.tile([128, 512], F32, tag="pd")  # [o_s; g_s]
                            nc.tensor.matmul(PA[:, :cw], lhsT=wsl(wt, 'lv_if'),
                                             rhs=XHv[:, c0:c0 + cw], start=True, stop=True)
                            nc.tensor.matmul(PB[:, :cw], lhsT=wsl(wt, 'lv_og'),
                                             rhs=XHv[:, c0:c0 + cw], start=True, stop=True)
                            nc.tensor.matmul(PC[:, :cw], lhsT=wsl(wt, 'ls_if'),
                                             rhs=XHs[:, c0:c0 + cw], start=True, stop=True)
                            nc.tensor.matmul(PD[:, :cw], lhsT=wsl(wt, 'ls_og'),
                                             rhs=XHs[:, c0:c0 + cw], start=True, stop=True)
                            # v-source: SA = sigmoid([i_v; f_v]) full 128
                            SA = wpA.tile([128, 512], F16, tag="SA")
                            nc.scalar.activation(SA[:, :cw], PA[:, :cw], AF.Sigmoid,
                                                 bias=bias('b_if_v'))
                            Sov = wpA.tile([64, 512], F16, tag="Sov")
                            nc.scalar.activation(Sov[:, :cw], PB[0:64, :cw], AF.Sigmoid,
                                                 bias=bias('b_o_v', 0, 64))
                            Tgv = wpA.tile([64, 512], F16, tag="Tgv")
                            nc.scalar.activation(Tgv[:, :cw], PB[64:128, :cw], AF.Tanh,
                                                 bias=bias('b_g_v', 0, 64))
                            # s-source split sigmoids (base-0 outputs)
                            Sis = wpA.tile([64, 512], F16, tag="Sis")
                            nc.scalar.activation(Sis[:, :cw], PC[0:64, :cw], AF.Sigmoid,
                                                 bias=bias('b_i_s', 0, 64))
                            Sfs = wpA.tile([64, 512], F16, tag="Sfs")
                            nc.scalar.activation(Sfs[:, :cw], PC[64:128, :cw], AF.Sigmoid,
                                                 bias=bias('b_f_s', 0, 64))
                            Sos = wpA.tile([64, 512], F16, tag="Sos")
                            nc.scalar.activation(Sos[:, :cw], PD[0:64, :cw], AF.Sigmoid,
                                                 bias=bias('b_o_s', 0, 64))
                            Tgs = wpA.tile([64, 512], F16, tag="Tgs")
                            nc.scalar.activation(Tgs[:, :cw], PD[64:128, :cw], AF.Tanh,
                                                 bias=bias('b_g_s', 0, 64))
                            # c_v (rows 64:128 of Cst): u = f_v*c_v ; v = i_v*g_v
                            uv = wpA.tile([64, 512], F32, tag="uv")
                            nc.vector.scalar_tensor_tensor(uv[:, :cw], SA[64:128, :cw], 0.0,
                                                           Cst[64:128, c0:c0 + cw],
                                                           op0=A.bypass, op1=A.mult)
                            vv = wpA.tile([64, 512], F32, tag="vv")
                            nc.vector.scalar_tensor_tensor(vv[:, :cw], SA[0:64, :cw], 0.0,
                                                           Tgv[:, :cw], op0=A.bypass, op1=A.mult)
                            nc.vector.scalar_tensor_tensor(Cst[64:128, c0:c0 + cw], uv[:, :cw],
                                                           0.0, vv[:, :cw],
                                                           op0=A.bypass, op1=A.add)
                            tcv = wpA.tile([64, 512], F32, tag="tcv")
                            nc.scalar.activation(tcv[:, :cw], Cst[64:128, c0:c0 + cw], AF.Tanh)
                            nc.vector.scalar_tensor_tensor(XHv[0:64, c0:c0 + cw], Sov[:, :cw],
                                                           0.0, tcv[:, :cw],
                                                           op0=A.bypass, op1=A.mult)
                            # c_s (rows 0:64 of Cst)
                            us = wpA.tile([64, 512], F32, tag="us")
                            nc.vector.scalar_tensor_tensor(us[:, :cw], Sfs[:, :cw], 0.0,
                                                           Cst[0:64, c0:c0 + cw],
                                                           op0=A.bypass, op1=A.mult)
                            vs = wpA.tile([64, 512], F32, tag="vs")
                            nc.vector.scalar_tensor_tensor(vs[:, :cw], Sis[:, :cw], 0.0,
                                                           Tgs[:, :cw], op0=A.bypass, op1=A.mult)
                            nc.vector.scalar_tensor_tensor(Cst[0:64, c0:c0 + cw], us[:, :cw],
                                                           0.0, vs[:, :cw],
                                                           op0=A.bypass, op1=A.add)
                            tcs = wpA.tile([64, 512], F32, tag="tcs")
                            nc.scalar.activation(tcs[:, :cw], Cst[0:64, c0:c0 + cw], AF.Tanh)
                            nc.vector.scalar_tensor_tensor(XHs[64:128, c0:c0 + cw], Sos[:, :cw],
                                                           0.0, tcs[:, :cw],
                                                           op0=A.bypass, op1=A.mult)

                # ---- Phase A epilogue: head + inline transposes/messages
                PT = stA.tile([32, NLP], F16)
                nc.sync.dma_start(PT[:], inputs['pers'][:])
                with (tc.tile_pool(name="psB", bufs=2, space="PSUM") as psB,
                      tc.tile_pool(name="psBs", bufs=2, space="PSUM") as psBs):
                    for (c0, cw) in CHUNKS:
                        pxa = psB.tile([128, 512], F32, tag="pa")
                        nc.tensor.matmul(pxa[:, :cw], lhsT=wsl(wt, 'lin'),
                                         rhs=XHv[0:64, c0:c0 + cw], start=True, stop=True)
                        pxp = psB.tile([128, 512], F32, tag="pb")
                        nc.tensor.matmul(pxp[:, :cw], lhsT=wsl(wt, 'pers'),
                                         rhs=PT[:, c0:c0 + cw], start=True, stop=True)
                        pxs = psB.tile([128, 512], F32, tag="pc")
                        nc.tensor.matmul(pxs[:, :cw], lhsT=wsl(wt, 'lin1'),
                                         rhs=XHs[64:128, c0:c0 + cw], start=True, stop=True)
                        XA = wpA.tile([128, 512], F16, tag="XA")
                        XP = wpA.tile([128, 512], F16, tag="XP")
                        XS = wpA.tile([128, 512], F16, tag="XS")
                        nc.scalar.activation(XA[:, :cw], pxa[:, :cw], AF.Lrelu, bias=bias('b_lin'), alpha=0.01)
                        nc.scalar.activation(XP[:, :cw], pxp[:, :cw], AF.Lrelu, bias=bias('b_pers'), alpha=0.01)
                        nc.scalar.activation(XS[:, :cw], pxs[:, :cw], AF.Lrelu, bias=bias('b_lin1'), alpha=0.01)
                        p0 = psB.tile([128, 512], F32, tag="pa")
                        nc.tensor.matmul(p0[:, :cw], lhsT=wsl(wt, 'lin2a'), rhs=XA[:, :cw], start=True, stop=False)
                        nc.tensor.matmul(p0[:, :cw], lhsT=wsl(wt, 'lin2b'), rhs=XP[:, :cw], start=False, stop=True)
                        p1 = psB.tile([128, 512], F32, tag="pb")
                        nc.tensor.matmul(p1[:, :cw], lhsT=wsl(wt, 'lin3a'), rhs=XS[:, :cw], start=True, stop=False)
                        nc.tensor.matmul(p1[:, :cw], lhsT=wsl(wt, 'lin3b'), rhs=XP[:, :cw], start=False, stop=True)
                        p2 = psB.tile([128, 512], F32, tag="pc")
                        nc.tensor.matmul(p2[:, :cw], lhsT=wsl(wt, 'lin4a'), rhs=XA[:, :cw], start=True, stop=False)
                        nc.tensor.matmul(p2[:, :cw], lhsT=wsl(wt, 'lin4bd'), rhs=XP[:, :cw], start=False, stop=False)
                        nc.tensor.matmul(p2[:, :cw], lhsT=wsl(wt, 'lin4c'), rhs=XS[:, :cw], start=False, stop=True)
                        Xc = [wpA.tile([128, 512], F16, tag=f"X{r}c", name=f"Xc{r}") for r in range(R)]
                        nc.scalar.activation(Xc[0][:, :cw], p0[:, :cw], AF.Lrelu, bias=bias('b2'), alpha=0.01)
                        nc.scalar.activation(Xc[1][:, :cw], p1[:, :cw], AF.Lrelu, bias=bias('b3'), alpha=0.01)
                        nc.scalar.activation(Xc[2][:, :cw], p2[:, :cw], AF.Lrelu, bias=bias('b4'), alpha=0.01)
                        for r in range(R):
                            nc.sync.dma_start(xin_dram[r][:, c0:c0 + cw], Xc[r][:, :cw])
                        p6 = psBs.tile([2, 512], F32, tag="p6")
                        nc.tensor.matmul(p6[:, :cw], lhsT=wsl(wt, 'lin6_3'), rhs=Xc[0][:, :cw],
                                         start=True, stop=False)
                        nc.tensor.matmul(p6[:, :cw], lhsT=wsl(wt, 'lin6_4'), rhs=Xc[1][:, :cw],
                                         start=False, stop=False)
                        nc.tensor.matmul(p6[:, :cw], lhsT=wsl(wt, 'lin6_5'), rhs=Xc[2][:, :cw],
                                         start=False, stop=True)
                        o6 = wpA.tile([2, 512], F32, tag="o6")
                        nc.scalar.copy(o6[:, :cw], p6[:, :cw])
                        nc.sync.dma_start(out_parts[3, :, c0:c0 + cw], o6[:, :cw])

            # transposes -> F0 (node-major) + hop-0 messages + hop-0 AllGathers
            with (tc.tile_pool(name="psT", bufs=2, space="PSUM") as psT,
                  tc.tile_pool(name="wpT", bufs=2) as wpT):
                for r in range(R):
                    for t in range(NT):
                        xt = wpT.tile([128, 128], F16, tag="xt")
                        nc.sync.dma_start(xt[:], xin_dram[r][:, t * 128:(t + 1) * 128])
                        tr = psT.tile([128, 128], F16, tag="tr")
                        nc.tensor.transpose(tr[:], xt[:], wsl(wt, 'ident'))
                        nc.vector.tensor_copy(FA[r][:, t * 128:(t + 1) * 128], tr[:])
                        m1 = wpT.tile([128, 128], F16, tag="m1")
                        nc.vector.tensor_scalar_mul(m1[:], tr[:], dv(r, t))
                        nc.sync.dma_start(ml[r][0][t * 128:(t + 1) * 128, :], m1[:])
                    nc.gpsimd.collective_compute(
                        "AllGather", A.bypass,
                        replica_groups=[list(range(NCORES))],
                        ins=[ml[r][0].opt()], outs=[mf[r][0].opt()],
                    )

            # =============== Phase B ===============
            def process_hop(r, hop):
                relm = meta['rel'][r]
                lo_w, hi_w = relm['lo_w'], relm['hi_w']
                lo_off, hi_off = [0], [0]
                for t in range(NT):
                    lo_off.append(lo_off[-1] + lo_w[t])
                    hi_off.append(hi_off[-1] + hi_w[t])
                maxlo = max(sum(lo_w[t0:t0 + GS]) for t0 in range(0, NT, GS))
                maxhi = max(sum(hi_w[t0:t0 + GS]) for t0 in range(0, NT, GS))
                fsrc = FA[r] if hop == 0 else FB[r]
                fdst = FB[r] if hop == 0 else FA[r]
                with (tc.tile_pool(name=f"psG{r}{hop}", bufs=2, space="PSUM") as psG,
                      tc.tile_pool(name=f"wpH{r}{hop}", bufs=2) as wpH):
                    for t0g in range(0, NT, GS):
                        tiles = list(range(t0g, min(t0g + GS, NT)))
                        nlo = sum(lo_w[t] for t in tiles)
                        nhi = sum(hi_w[t] for t in tiles)
                        bufs = {}
                        for cls, nwin, mx, woff in (('lo', nlo, maxlo, lo_off[tiles[0]]),
                                                    ('hi', nhi, maxhi, hi_off[tiles[0]])):
                            it = wpH.tile([128, mx * 8], I16, tag=f"idx{cls}")
                            nc.sync.dma_start(it[:, :nwin * 8],
                                              inputs[f'gidx_{cls}_{r}'][:, woff * 8:(woff + nwin) * 8])
                            gb = wpH.tile([128, mx, 128], F16, tag=f"gb{cls}")
                            mfh = mf[r][hop]
                            in_ap = mfh[0:LO_LIM, :] if cls == 'lo' else mfh[LO_LIM:NGP, :]
                            GW = 8
                            for w0 in range(0, nwin, GW):
                                sw = min(GW, nwin - w0)
                                nc.gpsimd.dma_gather(
                                    out_ap=gb[:, w0:w0 + sw, :], in_ap=in_ap,
                                    idxs_ap=it[:, w0 * 8:(w0 + sw) * 8],
                                    num_idxs=sw * 128, num_idxs_reg=sw * 128,
                                    elem_size=H)
                            ib = wpH.tile([128, mx, 128], F16, tag=f"ib{cls}")
                            nc.sync.dma_start(
                                ib[:, :nwin, :],
                                inputs[f'ind_{cls}_{r}'].ap()[woff * 128:(woff + nwin) * 128, :]
                                .rearrange("(w e) d -> e w d", w=nwin))
                            bufs[cls] = (gb, ib)
                        for t in tiles:
                            agg = psG.tile([128, 128], F32, tag="agg")
                            wins = ([('lo', lo_off[t] - lo_off[tiles[0]] + w) for w in range(lo_w[t])]
                                    + [('hi', hi_off[t] - hi_off[tiles[0]] + w) for w in range(hi_w[t])])
                            for wi, (cls, w) in enumerate(wins):
                                gb, ib = bufs[cls]
                                nc.tensor.matmul(agg[:], lhsT=ib[:, w, :], rhs=gb[:, w, :],
                                                 start=(wi == 0), stop=(wi == len(wins) - 1))
                            nc.vector.scalar_tensor_tensor(
                                fdst[:, t * 128:(t + 1) * 128], agg[:], ndv(r, t),
                                fsrc[:, t * 128:(t + 1) * 128],
                                op0=A.mult, op1=A.add)
                            if hop == 0:
                                m1 = wpH.tile([128, 128], F16, tag="m1")
                                nc.vector.tensor_scalar_mul(
                                    m1[:], fdst[:, t * 128:(t + 1) * 128], dv(r, t))
                                nc.sync.dma_start(ml[r][1][t * 128:(t + 1) * 128, :], m1[:])
                if hop == 0:
                    nc.gpsimd.collective_compute(
                        "AllGather", A.bypass,
                        replica_groups=[list(range(NCORES))],
                        ins=[ml[r][1].opt()], outs=[mf[r][1].opt()],
                    )

            def epilogue(r):
                # F1 = FB[r], F2 = FA[r] (node-major fp16); x_in reloaded from DRAM
                with (tc.tile_pool(name=f"psEt{r}", bufs=2, space="PSUM") as psEt,
                      tc.tile_pool(name=f"psEw{r}", bufs=1, space="PSUM") as psEw,
                      tc.tile_pool(name=f"wpE{r}", bufs=2) as wpE):
                    for (c0, cw) in CHUNKS:
                        nsub = cw // 128
                        X0c = wpE.tile([128, 512], F16, tag="X0c")
                        nc.sync.dma_start(X0c[:, :cw], xin_dram[r][:, c0:c0 + cw])
                        F1c = wpE.tile([128, 512], F16, tag="F1c")
                        F2c = wpE.tile([128, 512], F16, tag="F2c")
                        for si in range(nsub):
                            tr = psEt.tile([128, 128], F16, tag="tr")
                            nc.tensor.transpose(tr[:], FB[r][:, c0 + si * 128:c0 + (si + 1) * 128],
                                                wsl(wt, 'ident'))
                            nc.vector.tensor_copy(F1c[:, si * 128:(si + 1) * 128], tr[:])
                            tr2 = psEt.tile([128, 128], F16, tag="tr")
                            nc.tensor.transpose(tr2[:], FA[r][:, c0 + si * 128:c0 + (si + 1) * 128],
                                                wsl(wt, 'ident'))
                            nc.vector.tensor_copy(F2c[:, si * 128:(si + 1) * 128], tr2[:])
                        den_sb = wpE.tile([1, 512], F32, tag="den")
                        gsb = [wpE.tile([1, 512], F32, tag=f"gs{j}", name=f"gsb{j}")
                               for j in range(3)]
                        Bsrc = [X0c[:, :cw], F1c[:, :cw], F2c[:, :cw]]
                        for o in range(5):
                            pso = psEw.tile([128, 512], F32, tag="to")
                            js = [j for j in range(3) if CTRUE[o][j] != 0.0]
                            for ji, j in enumerate(js):
                                nc.tensor.matmul(pso[:, :cw], lhsT=wsl(wt, f'wf1_{r}_{o}_{j}'),
                                                 rhs=Bsrc[j], start=(ji == 0), stop=(ji == len(js) - 1))
                            To = wpE.tile([128, 512], F16, tag="To")
                            nc.scalar.activation(To[:, :cw], pso[:, :cw], AF.Tanh, bias=bias(f'bf1_{r}'))
                            psc = psEw.tile([1, 512], F32, tag="sc")
                            nc.tensor.matmul(psc[:, :cw], lhsT=wsl(wt, f'wf2_{r}'), rhs=To[:, :cw],
                                             start=True, stop=True)
                            eo = wpE.tile([1, 512], F16, tag="eo")
                            nc.scalar.activation(eo[:, :cw], psc[:, :cw], AF.Exp)
                            if o == 0:
                                nc.vector.tensor_copy(den_sb[:, :cw], eo[:, :cw])
                                for j in range(3):
                                    nc.vector.tensor_scalar_mul(gsb[j][:, :cw], eo[:, :cw],
                                                                float(CTRUE[o][j]))
                            else:
                                nc.vector.scalar_tensor_tensor(den_sb[:, :cw], eo[:, :cw], 0.0,
                                                               den_sb[:, :cw],
                                                               op0=A.bypass, op1=A.add)
                                for j in range(3):
                                    if CTRUE[o][j] != 0.0:
                                        nc.vector.scalar_tensor_tensor(
                                            gsb[j][:, :cw], eo[:, :cw], float(CTRUE[o][j]),
                                            gsb[j][:, :cw], op0=A.mult, op1=A.add)
                        rec = wpE.tile([1, 512], F32, tag="rec")
                        nc.vector.reciprocal(rec[:, :cw], den_sb[:, :cw])
                        res = wpE.tile([128, 512], F16, tag="res")
                        tmp = wpE.tile([128, 512], F16, tag="tmp")
                        for j in range(3):
                            gj = wpE.tile([1, 512], F16, tag="gj")
                            nc.vector.scalar_tensor_tensor(gj[:, :cw], rec[:, :cw], 0.0,
                                                           gsb[j][:, :cw], op0=A.bypass, op1=A.mult)
                            pbj = psEw.tile([128, 512], F32, tag="bc")
                            nc.tensor.matmul(pbj[:, :cw], lhsT=onesf16[:], rhs=gj[:, :cw],
                                             start=True, stop=True)
                            if j == 0:
                                nc.vector.scalar_tensor_tensor(res[:, :cw], Bsrc[j], 0.0, pbj[:, :cw],
                                                               op0=A.bypass, op1=A.mult)
                            else:
                                nc.vector.scalar_tensor_tensor(tmp[:, :cw], Bsrc[j], 0.0, pbj[:, :cw],
                                                               op0=A.bypass, op1=A.mult)
                                nc.vector.scalar_tensor_tensor(res[:, :cw], res[:, :cw], 0.0,
                                                               tmp[:, :cw], op0=A.bypass, op1=A.add)
                        ph = psEw.tile([128, 512], F32, tag="to")
                        nc.tensor.matmul(ph[:, :cw], lhsT=wsl(wt, f'lin5_{r}'), rhs=res[:, :cw],
                                         start=True, stop=True)
                        hall = wpE.tile([128, 512], F16, tag="hall")
                        nc.scalar.activation(hall[:, :cw], ph[:, :cw], AF.Lrelu,
                                             bias=bias(f'b5_{r}'), alpha=0.01)
                        po = psEw.tile([2, 512], F32, tag="po")
                        nc.tensor.matmul(po[:, :cw], lhsT=wsl(wt, f'lin6_{r}'), rhs=hall[:, :cw],
                                         start=True, stop=True)
                        oo = wpE.tile([2, 512], F32, tag="oo")
                        nc.scalar.copy(oo[:, :cw], po[:, :cw])
                        nc.sync.dma_start(out_parts[r, :, c0:c0 + cw], oo[:, :cw])

            # software pipeline: hop0 x3 (each retriggers its AG for hop1),
            # then hop1 + epilogue per relation
            import os as _os
            _KPART = _os.environ.get("KPART", "ALL")
            if _KPART != "A":
                for r in range(R if _KPART in ("ALL", "H1") else 1):
                    process_hop(r, 0)
            if _KPART in ("ALL", "H1"):
                for r in range(R):
                    process_hop(r, 1)
                    if _KPART == "ALL":
                        epilogue(r)

    nc.compile()


def kernel(**inp):
    meta, cores = _prep(inp)
    nc = bacc.Bacc("TRN2", target_bir_lowering=False, debug=False, num_devices=NCORES)
    _build(nc, meta)
    res = run_bass_kernel_spmd(nc, [dict(c) for c in cores], core_ids=list(range(NCORES)))
    out = np.zeros((N, C), np.float32)
    b6 = np.asarray(inp['b_lin6'], np.float32)
    for c in range(NCORES):
        parts = res.results[c]["out_parts"]
        out[c * NL:(c + 1) * NL] = parts.sum(axis=0).T[:NL] + b6[None, :]
    return out


if __name__ == "__main__":
    import reference
    inputs = {k: np.asarray(v) for k, v in reference.setup_inputs().items()}
    got = kernel(**inputs)
    exp = np.asarray(reference.reference(**inputs))
    err = np.abs(got - exp).max()
    rel = err / max(np.abs(exp).max(), 1e-9)
    print("abs err:", err, "rel err:", rel)


# revision 29
# speedup vs baseline: 1.2915x; 1.0649x over previous
"""BWGNN_Hetero Trainium2 kernel: 8-core SPMD, node-sharded graph/data parallel.

v2: fp16 tensor-engine path (merged M=128 LSTM gate matmuls, fp16 weights/
activations, fp16 feature upload), software-pipelined Phase B (all three
relations' hop-0 AllGathers issued at the end of Phase A; hops interleaved
across relations so collectives and tensor epilogues hide under the gpsimd
dma_gather stream), single fused attention epilogue pass, and one dma_gather
call per (tile-group, class).

Algorithmic reduction: the 5 beta-wavelet filters are linear combinations of
{f0, f1=L f0, f2=L f1} (L = normalized Laplacian), so each relation needs
only 2 sparse hops.
"""
import sys
sys.path.insert(0, '/opt/trn_rl_repo')
sys.path.insert(0, '/root/problem')

import numpy as np

import concourse.bacc as bacc
import concourse.bass as bass
import concourse.mybir as mybir
import concourse.tile as tile
from concourse.bass_utils import run_bass_kernel_spmd

F32 = mybir.dt.float32
F16 = mybir.dt.float16
I16 = mybir.dt.int16
A = mybir.AluOpType
AF = mybir.ActivationFunctionType

NCORES = 8
N, E, R, T = 50000, 800000, 3, 16
IV, IS, IP, H, C = 64, 64, 32, 128, 2
NL = N // NCORES            # 6250
NT = 49                     # dst tiles per core
NLP = NT * 128              # 6272 padded local nodes
NGP = NLP * NCORES          # 50176 padded global rows in AllGather output
LO_LIM = 32768              # int16 gather index limit
GS = 2                      # dst tiles per gather group
CHUNKS = [(i * 512, 512) for i in range(12)] + [(6144, 128)]

CTRUE = [[0.8, -0.5, 0.0],
         [3.0, -3.0, 0.75],
         [0.0, 3.0, -1.5],
         [0.0, 0.0, 0.75],
         [-0.2, 0.5, 0.0]]


def _wrap_idx(idx):
    """[n] int16 -> [128, ceil(n/16)] wrapped (i -> [i%16, i//16]) + replicated x8."""
    n = len(idx)
    L = max(1, (n + 15) // 16)
    a = np.zeros((16, L), np.int16)
    for p in range(16):
        vals = idx[p::16]
        a[p, :len(vals)] = vals
    return np.tile(a, (8, 1))


class WPack:
    def __init__(self, dtype):
        self.dtype = dtype
        self.cols = []
        self.off = 0
        self.slots = {}

    def add(self, name, mat, row0=0):
        mat = np.asarray(mat, self.dtype)
        k, m = mat.shape
        assert row0 + k <= 128
        buf = np.zeros((128, m), self.dtype)
        buf[row0:row0 + k] = mat
        self.cols.append(buf)
        self.slots[name] = (row0, k, self.off, m)
        self.off += m

    def image(self):
        return np.concatenate(self.cols, axis=1)


def _prep(inp):
    g = {k: np.asarray(v) for k, v in inp.items()}
    wp = WPack(np.float16)

    # torch gate-row offsets: i=0, f=64, g=128, o=192
    # XHv rows: [h_v (0:64); x_v (64:128)];  XHs rows: [x_s (0:64); h_s (64:128)]
    def lv_pair(g1, g2):
        m = np.zeros((128, 128), np.float32)
        for ci, ro in enumerate((g1, g2)):
            m[0:64, ci * 64:(ci + 1) * 64] = g['Whh_v'][ro:ro + 64, :].T
            m[64:128, ci * 64:(ci + 1) * 64] = g['Wih_v'][ro:ro + 64, :].T
        return m

    def ls_pair(g1, g2):
        m = np.zeros((128, 128), np.float32)
        for ci, ro in enumerate((g1, g2)):
            m[0:64, ci * 64:(ci + 1) * 64] = g['Wih_s'][ro:ro + 64, :].T
            m[64:128, ci * 64:(ci + 1) * 64] = g['Whh_s'][ro:ro + 64, :].T
        return m

    wp.add('lv_if', lv_pair(0, 64))
    wp.add('lv_og', lv_pair(192, 128))
    wp.add('ls_if', ls_pair(0, 64))
    wp.add('ls_og', ls_pair(192, 128))
    wp.add('lin', g['W_lin'].T)                       # rows 0:64 (rhs = h_v at base 0)
    wp.add('lin1', g['W_lin1'].T, row0=64)            # rows 64:128 (rhs = h_s at base 64)
    wp.add('pers', g['W_pers'].T)
    wp.add('lin2a', g['W_lin2'][:, 0:128].T)
    wp.add('lin2b', g['W_lin2'][:, 128:256].T)
    wp.add('lin3a', g['W_lin3'][:, 0:128].T)
    wp.add('lin3b', g['W_lin3'][:, 128:256].T)
    wp.add('lin4a', g['W_lin4'][:, 0:128].T)
    wp.add('lin4bd', (g['W_lin4'][:, 128:256] + g['W_lin4'][:, 384:512]).T)
    wp.add('lin4c', g['W_lin4'][:, 256:384].T)
    for r in range(R):
        for o in range(5):
            for j in range(3):
                if CTRUE[o][j] != 0.0:
                    wp.add(f'wf1_{r}_{o}_{j}', (CTRUE[o][j] * g['Wf1'][r]).T)
        wp.add(f'wf2_{r}', g['Wf2'][r][:, None])
        wp.add(f'lin5_{r}', g['W_lin5'][r].T)
    for k in range(6):
        wp.add(f'lin6_{k}', g['W_lin6'][:, k * 128:(k + 1) * 128].T)
    wp.add('ident', np.eye(128, dtype=np.float32))
    wp.add('one11', np.ones((1, 1), np.float32))
    for o in range(5):
        for j in range(3):
            wp.add(f'c_{o}_{j}', np.array([[CTRUE[o][j]]], np.float32))
    wimg = wp.image()

    bcols, blist = {}, []

    def addb(name, vec):
        bcols[name] = len(blist)
        v = np.zeros((128, 1), np.float32)
        v[:len(vec), 0] = np.asarray(vec, np.float32).ravel()
        blist.append(v)

    bv = g['bih_v'] + g['bhh_v']
    bs = g['bih_s'] + g['bhh_s']
    addb('b_if_v', np.concatenate([bv[0:64], bv[64:128]]))     # [i_v; f_v]
    addb('b_o_v', bv[192:256])
    addb('b_g_v', bv[128:192])
    addb('b_i_s', bs[0:64])
    addb('b_f_s', bs[64:128])
    addb('b_o_s', bs[192:256])
    addb('b_g_s', bs[128:192])
    addb('b_lin', g['b_lin'])
    addb('b_lin1', g['b_lin1'])
    addb('b_pers', g['b_pers'])
    addb('b2', g['b_lin2'])
    addb('b3', g['b_lin3'])
    addb('b4', g['b_lin4'])
    for r in range(R):
        addb(f'bf1_{r}', g['bf1'][r])
        addb(f'b5_{r}', g['b_lin5'][r])
    bimg = np.concatenate(blist, axis=1)

    src = np.asarray(g['src'], np.int64)
    dst = np.asarray(g['dst'], np.int64)
    gsrc_all = (src // NL) * NLP + (src % NL)

    percore = [dict() for _ in range(NCORES)]
    relmeta = []
    for r in range(R):
        deg = np.bincount(dst[r], minlength=N).astype(np.float32)
        dinv = np.clip(deg, 1.0, None) ** -0.5

        # bucket edges: (core, tile, class)
        per = []
        for c in range(NCORES):
            m = (dst[r] // NL) == c
            sc = gsrc_all[r][m]
            dl = dst[r][m] - c * NL
            tl, col = dl // 128, dl % 128
            tiles = []
            for t in range(NT):
                mt = tl == t
                st_, ct_ = sc[mt], col[mt]
                lo = st_ < LO_LIM
                tiles.append((st_[lo], ct_[lo], st_[~lo] - LO_LIM, ct_[~lo]))
            per.append(tiles)
        # common (max-over-cores) window counts
        lo_w = [max(1, max((len(per[c][t][0]) + 127) // 128 for c in range(NCORES)))
                for t in range(NT)]
        hi_w = [max(1, max((len(per[c][t][2]) + 127) // 128 for c in range(NCORES)))
                for t in range(NT)]
        relmeta.append({'lo_w': lo_w, 'hi_w': hi_w})
        for c in range(NCORES):
            li_s, lc_s, hi_s, hc_s = [], [], [], []
            for t in range(NT):
                li, lc, hi, hc = per[c][t]
                lp = np.zeros(lo_w[t] * 128, np.int64); lp[:len(li)] = li
                lcp = np.full(lo_w[t] * 128, -1, np.int64); lcp[:len(lc)] = lc
                hp = np.zeros(hi_w[t] * 128, np.int64); hp[:len(hi)] = hi
                hcp = np.full(hi_w[t] * 128, -1, np.int64); hcp[:len(hc)] = hc
                li_s.append(lp); lc_s.append(lcp); hi_s.append(hp); hc_s.append(hcp)
            li_s = np.concatenate(li_s); lc_s = np.concatenate(lc_s)
            hi_s = np.concatenate(hi_s); hc_s = np.concatenate(hc_s)

            def mkind(colarr):
                # wrapped layout [e, w*128 + c]: partition = edge-in-window, so the
                # device DMA is a contiguous per-partition copy (no 256B scatter)
                W = len(colarr) // 128
                ind = np.zeros((128, W * 128), np.float16)
                valid = colarr >= 0
                pos = np.nonzero(valid)[0]
                w, e = pos // 128, pos % 128
                ind[e, w * 128 + colarr[valid]] = 1.0
                return ind
            pc = percore[c]
            pc[f'gidx_lo_{r}'] = _wrap_idx(li_s.astype(np.int16))
            pc[f'gidx_hi_{r}'] = _wrap_idx(hi_s.astype(np.int16))
            pc[f'ind_lo_{r}'] = mkind(lc_s)
            pc[f'ind_hi_{r}'] = mkind(hc_s)
            dp = pc.setdefault('_dinv', np.zeros((128, 2 * R * NT), np.float32))
            dvl = np.ones(NLP, np.float32)
            dvl[:NL] = dinv[c * NL:(c + 1) * NL]
            dp[:, r * NT:(r + 1) * NT] = dvl.reshape(NT, 128).T
            dp[:, R * NT + r * NT:R * NT + (r + 1) * NT] = -dvl.reshape(NT, 128).T

    voc = np.asarray(g['voc_features'], np.float16)
    sms = np.asarray(g['sms_features'], np.float16)
    pers = np.asarray(g['personal_feature'], np.float16)
    cores = []
    for c in range(NCORES):
        pc = percore[c]
        sl = slice(c * NL, (c + 1) * NL)
        vt = np.zeros((T, IV, NLP), np.float16)
        st_ = np.zeros((T, IS, NLP), np.float16)
        vt[:, :, :NL] = voc[sl].transpose(1, 2, 0)
        st_[:, :, :NL] = sms[sl].transpose(1, 2, 0)
        pt = np.zeros((IP, NLP), np.float16)
        pt[:, :NL] = pers[sl].T
        pc['voc'] = vt
        pc['sms'] = st_
        pc['pers'] = pt
        pc['wpack'] = wimg
        pc['bpack'] = bimg
        pc['dpack'] = pc.pop('_dinv')
        cores.append(pc)
    meta = {
        'wp': wp.slots, 'bcols': bcols, 'rel': relmeta,
        'shapes': {k: v.shape for k, v in cores[0].items()},
        'dtypes': {k: v.dtype for k, v in cores[0].items()},
    }
    return meta, cores


def _build(nc, meta):
    sh, dt = meta['shapes'], meta['dtypes']
    WP, BC = meta['wp'], meta['bcols']
    inputs = {k: nc.dram_tensor(k, list(sh[k]), mybir.dt.from_np(np.dtype(dt[k])),
                                kind="ExternalInput") for k in sh}
    out_parts = nc.dram_tensor("out_parts", [4, 2, NLP], F32, kind="ExternalOutput")

    def wsl(wt, name):
        r0, k, off, m = WP[name]
        return wt[r0:r0 + k, off:off + m]

    with tile.TileContext(nc) as tc:
        with (
            tc.tile_pool(name="const", bufs=1) as cpool,
            tc.tile_pool(name="persist", bufs=1) as spool,
            tc.tile_pool(name="dram", bufs=1, space="DRAM") as dpool,
        ):
            wt = cpool.tile([128, sh['wpack'][1]], F16)
            nc.sync.dma_start(wt[:], inputs['wpack'][:])
            bt = cpool.tile([128, sh['bpack'][1]], F32)
            nc.sync.dma_start(bt[:], inputs['bpack'][:])
            dpt = cpool.tile([128, 2 * R * NT], F32)
            nc.sync.dma_start(dpt[:], inputs['dpack'][:])
            onesf16 = cpool.tile([1, 128], F16)
            nc.vector.memset(onesf16[:], 1.0)

            def bias(name, p0=0, pn=128):
                return bt[p0:p0 + pn, BC[name]:BC[name] + 1]

            # persistent fp16 node-major filter states: FA_r (f0, later f2), FB_r (f1)
            FA = [spool.tile([128, NLP], F16, name=f"FA{r}") for r in range(R)]
            FB = [spool.tile([128, NLP], F16, name=f"FB{r}") for r in range(R)]
            xin_dram = [dpool.tile([128, NLP], F16, tag=f"xin{r}", name=f"xind{r}")
                        for r in range(R)]
            ml = [[dpool.tile([NLP, H], F16, tag=f"ml{r}h{h}", name=f"mld{r}h{h}")
                   for h in range(2)] for r in range(R)]
            mf = [[dpool.tile([NGP, H], F16, tag=f"mf{r}h{h}", name=f"mfd{r}h{h}",
                              addr_space="Shared") for h in range(2)] for r in range(R)]

            def dv(r, t):
                return dpt[:, r * NT + t:r * NT + t + 1]

            def ndv(r, t):
                return dpt[:, R * NT + r * NT + t:R * NT + r * NT + t + 1]

            # =============== Phase A: fused LSTMs ===============
            with (tc.tile_pool(name="stA", bufs=1) as stA,
                  tc.tile_pool(name="wpA", bufs=2) as wpA):
                XHv = stA.tile([128, NLP], F16)
                XHs = stA.tile([128, NLP], F16)
                Cst = stA.tile([128, NLP], F32)   # rows 64:128 = c_v, rows 0:64 = c_s
                nc.vector.memset(XHv[0:64, :], 0.0)
                nc.vector.memset(XHs[64:128, :], 0.0)
                nc.vector.memset(Cst[:], 0.0)
                with tc.tile_pool(name="psA", bufs=2, space="PSUM") as psA:
                    for t in range(T):
                        nc.sync.dma_start(XHv[64:128, :], inputs['voc'][t])
                        nc.sync.dma_start(XHs[0:64, :], inputs['sms'][t])
                        for (c0, cw) in CHUNKS:
                            PA = psA.tile([128, 512], F32, tag="pa")  # [i_v; f_v]
                            PB = psA.tile([128, 512], F32, tag="pb")  # [o_v; g_v]
                            PC = psA.tile([128, 512], F32, tag="pc")  # [i_s; f_s]
                            PD = psA.tile([128, 512], F32, tag="pd")  # [o_s; g_s]
                            nc.tensor.matmul(PA[:, :cw], lhsT=wsl(wt, 'lv_if'),
                                             rhs=XHv[:, c0:c0 + cw], start=True, stop=True)
                            nc.tensor.matmul(PB[:, :cw], lhsT=wsl(wt, 'lv_og'),
                                             rhs=XHv[:, c0:c0 + cw], start=True, stop=True)
                            nc.tensor.matmul(PC[:, :cw], lhsT=wsl(wt, 'ls_if'),
                                             rhs=XHs[:, c0:c0 + cw], start=True, stop=True)
                            nc.tensor.matmul(PD[:, :cw], lhsT=wsl(wt, 'ls_og'),
                                             rhs=XHs[:, c0:c0 + cw], start=True, stop=True)
                            # v-source: SA = sigmoid([i_v; f_v]) full 128
                            SA = wpA.tile([128, 512], F16, tag="SA")
                            nc.scalar.activation(SA[:, :cw], PA[:, :cw], AF.Sigmoid,
                                                 bias=bias('b_if_v'))
                            Sov = wpA.tile([64, 512], F16, tag="Sov")
                            nc.scalar.activation(Sov[:, :cw], PB[0:64, :cw], AF.Sigmoid,
                                                 bias=bias('b_o_v', 0, 64))
                            Tgv = wpA.tile([64, 512], F16, tag="Tgv")
                            nc.scalar.activation(Tgv[:, :cw], PB[64:128, :cw], AF.Tanh,
                                                 bias=bias('b_g_v', 0, 64))
                            # s-source split sigmoids (base-0 outputs)
                            Sis = wpA.tile([64, 512], F16, tag="Sis")
                            nc.scalar.activation(Sis[:, :cw], PC[0:64, :cw], AF.Sigmoid,
                                                 bias=bias('b_i_s', 0, 64))
                            Sfs = wpA.tile([64, 512], F16, tag="Sfs")
                            nc.scalar.activation(Sfs[:, :cw], PC[64:128, :cw], AF.Sigmoid,
                                                 bias=bias('b_f_s', 0, 64))
                            Sos = wpA.tile([64, 512], F16, tag="Sos")
                            nc.scalar.activation(Sos[:, :cw], PD[0:64, :cw], AF.Sigmoid,
                                                 bias=bias('b_o_s', 0, 64))
                            Tgs = wpA.tile([64, 512], F16, tag="Tgs")
                            nc.scalar.activation(Tgs[:, :cw], PD[64:128, :cw], AF.Tanh,
                                                 bias=bias('b_g_s', 0, 64))
                            # c_v (rows 64:128 of Cst): u = f_v*c_v ; v = i_v*g_v
                            uv = wpA.tile([64, 512], F32, tag="uv")
                            nc.vector.scalar_tensor_tensor(uv[:, :cw], SA[64:128, :cw], 0.0,
                                                           Cst[64:128, c0:c0 + cw],
                                                           op0=A.bypass, op1=A.mult)
                            vv = wpA.tile([64, 512], F32, tag="vv")
                            nc.vector.scalar_tensor_tensor(vv[:, :cw], SA[0:64, :cw], 0.0,
                                                           Tgv[:, :cw], op0=A.bypass, op1=A.mult)
                            nc.vector.scalar_tensor_tensor(Cst[64:128, c0:c0 + cw], uv[:, :cw],
                                                           0.0, vv[:, :cw],
                                                           op0=A.bypass, op1=A.add)
                            tcv = wpA.tile([64, 512], F32, tag="tcv")
                            nc.scalar.activation(tcv[:, :cw], Cst[64:128, c0:c0 + cw], AF.Tanh)
                            nc.vector.scalar_tensor_tensor(XHv[0:64, c0:c0 + cw], Sov[:, :cw],
                                                           0.0, tcv[:, :cw],
                                                           op0=A.bypass, op1=A.mult)
                            # c_s (rows 0:64 of Cst)
                            us = wpA.tile([64, 512], F32, tag="us")
                            nc.vector.scalar_tensor_tensor(us[:, :cw], Sfs[:, :cw], 0.0,
                                                           Cst[0:64, c0:c0 + cw],
                                                           op0=A.bypass, op1=A.mult)
                            vs = wpA.tile([64, 512], F32, tag="vs")
                            nc.vector.scalar_tensor_tensor(vs[:, :cw], Sis[:, :cw], 0.0,
                                                           Tgs[:, :cw], op0=A.bypass, op1=A.mult)
                            nc.vector.scalar_tensor_tensor(Cst[0:64, c0:c0 + cw], us[:, :cw],
                                                           0.0, vs[:, :cw],
                                                           op0=A.bypass, op1=A.add)
                            tcs = wpA.tile([64, 512], F32, tag="tcs")
                            nc.scalar.activation(tcs[:, :cw], Cst[0:64, c0:c0 + cw], AF.Tanh)
                            nc.vector.scalar_tensor_tensor(XHs[64:128, c0:c0 + cw], Sos[:, :cw],
                                                           0.0, tcs[:, :cw],
                                                           op0=A.bypass, op1=A.mult)

                # ---- Phase A epilogue: head + inline transposes/messages
                PT = stA.tile([32, NLP], F16)
                nc.sync.dma_start(PT[:], inputs['pers'][:])
                with (tc.tile_pool(name="psB", bufs=2, space="PSUM") as psB,
                      tc.tile_pool(name="psBs", bufs=2, space="PSUM") as psBs):
                    for (c0, cw) in CHUNKS:
                        pxa = psB.tile([128, 512], F32, tag="pa")
                        nc.tensor.matmul(pxa[:, :cw], lhsT=wsl(wt, 'lin'),
                                         rhs=XHv[0:64, c0:c0 + cw], start=True, stop=True)
                        pxp = psB.tile([128, 512], F32, tag="pb")
                        nc.tensor.matmul(pxp[:, :cw], lhsT=wsl(wt, 'pers'),
                                         rhs=PT[:, c0:c0 + cw], start=True, stop=True)
                        pxs = psB.tile([128, 512], F32, tag="pc")
                        nc.tensor.matmul(pxs[:, :cw], lhsT=wsl(wt, 'lin1'),
                                         rhs=XHs[64:128, c0:c0 + cw], start=True, stop=True)
                        XA = wpA.tile([128, 512], F16, tag="XA")
                        XP = wpA.tile([128, 512], F16, tag="XP")
                        XS = wpA.tile([128, 512], F16, tag="XS")
                        nc.scalar.activation(XA[:, :cw], pxa[:, :cw], AF.Lrelu, bias=bias('b_lin'), alpha=0.01)
                        nc.scalar.activation(XP[:, :cw], pxp[:, :cw], AF.Lrelu, bias=bias('b_pers'), alpha=0.01)
                        nc.scalar.activation(XS[:, :cw], pxs[:, :cw], AF.Lrelu, bias=bias('b_lin1'), alpha=0.01)
                        p0 = psB.tile([128, 512], F32, tag="pa")
                        nc.tensor.matmul(p0[:, :cw], lhsT=wsl(wt, 'lin2a'), rhs=XA[:, :cw], start=True, stop=False)
                        nc.tensor.matmul(p0[:, :cw], lhsT=wsl(wt, 'lin2b'), rhs=XP[:, :cw], start=False, stop=True)
                        p1 = psB.tile([128, 512], F32, tag="pb")
                        nc.tensor.matmul(p1[:, :cw], lhsT=wsl(wt, 'lin3a'), rhs=XS[:, :cw], start=True, stop=False)
                        nc.tensor.matmul(p1[:, :cw], lhsT=wsl(wt, 'lin3b'), rhs=XP[:, :cw], start=False, stop=True)
                        p2 = psB.tile([128, 512], F32, tag="pc")
                        nc.tensor.matmul(p2[:, :cw], lhsT=wsl(wt, 'lin4a'), rhs=XA[:, :cw], start=True, stop=False)
                        nc.tensor.matmul(p2[:, :cw], lhsT=wsl(wt, 'lin4bd'), rhs=XP[:, :cw], start=False, stop=False)
                        nc.tensor.matmul(p2[:, :cw], lhsT=wsl(wt, 'lin4c'), rhs=XS[:, :cw], start=False, stop=True)
                        Xc = [wpA.tile([128, 512], F16, tag=f"X{r}c", name=f"Xc{r}") for r in range(R)]
                        nc.scalar.activation(Xc[0][:, :cw], p0[:, :cw], AF.Lrelu, bias=bias('b2'), alpha=0.01)
                        nc.scalar.activation(Xc[1][:, :cw], p1[:, :cw], AF.Lrelu, bias=bias('b3'), alpha=0.01)
                        nc.scalar.activation(Xc[2][:, :cw], p2[:, :cw], AF.Lrelu, bias=bias('b4'), alpha=0.01)
                        for r in range(R):
                            nc.sync.dma_start(xin_dram[r][:, c0:c0 + cw], Xc[r][:, :cw])
                        p6 = psBs.tile([2, 512], F32, tag="p6")
                        nc.tensor.matmul(p6[:, :cw], lhsT=wsl(wt, 'lin6_3'), rhs=Xc[0][:, :cw],
                                         start=True, stop=False)
                        nc.tensor.matmul(p6[:, :cw], lhsT=wsl(wt, 'lin6_4'), rhs=Xc[1][:, :cw],
                                         start=False, stop=False)
                        nc.tensor.matmul(p6[:, :cw], lhsT=wsl(wt, 'lin6_5'), rhs=Xc[2][:, :cw],
                                         start=False, stop=True)
                        o6 = wpA.tile([2, 512], F32, tag="o6")
                        nc.scalar.copy(o6[:, :cw], p6[:, :cw])
                        nc.sync.dma_start(out_parts[3, :, c0:c0 + cw], o6[:, :cw])

            # transposes -> F0 (node-major) + hop-0 messages + hop-0 AllGathers
            with (tc.tile_pool(name="psT", bufs=2, space="PSUM") as psT,
                  tc.tile_pool(name="wpT", bufs=2) as wpT):
                for r in range(R):
                    for t in range(NT):
                        xt = wpT.tile([128, 128], F16, tag="xt")
                        nc.sync.dma_start(xt[:], xin_dram[r][:, t * 128:(t + 1) * 128])
                        tr = psT.tile([128, 128], F16, tag="tr")
                        nc.tensor.transpose(tr[:], xt[:], wsl(wt, 'ident'))
                        nc.vector.tensor_copy(FA[r][:, t * 128:(t + 1) * 128], tr[:])
                        m1 = wpT.tile([128, 128], F16, tag="m1")
                        nc.vector.tensor_scalar_mul(m1[:], tr[:], dv(r, t))
                        nc.sync.dma_start(ml[r][0][t * 128:(t + 1) * 128, :], m1[:])
                    nc.gpsimd.collective_compute(
                        "AllGather", A.bypass,
                        replica_groups=[list(range(NCORES))],
                        ins=[ml[r][0].opt()], outs=[mf[r][0].opt()],
                    )

            # =============== Phase B ===============
            MAXLO = max(max(sum(meta['rel'][r]['lo_w'][t0:t0 + GS])
                            for t0 in range(0, NT, GS)) for r in range(R))
            MAXHI = max(max(sum(meta['rel'][r]['hi_w'][t0:t0 + GS])
                            for t0 in range(0, NT, GS)) for r in range(R))

            def process_hop(r, hop, psG, wpH):
                relm = meta['rel'][r]
                lo_w, hi_w = relm['lo_w'], relm['hi_w']
                lo_off, hi_off = [0], [0]
                for t in range(NT):
                    lo_off.append(lo_off[-1] + lo_w[t])
                    hi_off.append(hi_off[-1] + hi_w[t])
                fsrc = FA[r] if hop == 0 else FB[r]
                fdst = FB[r] if hop == 0 else FA[r]
                if True:
                    for t0g in range(0, NT, GS):
                        tiles = list(range(t0g, min(t0g + GS, NT)))
                        nlo = sum(lo_w[t] for t in tiles)
                        nhi = sum(hi_w[t] for t in tiles)
                        bufs = {}
                        for cls, nwin, mx, woff in (('lo', nlo, MAXLO, lo_off[tiles[0]]),
                                                    ('hi', nhi, MAXHI, hi_off[tiles[0]])):
                            it = wpH.tile([128, mx * 8], I16, tag=f"idx{cls}")
                            nc.sync.dma_start(it[:, :nwin * 8],
                                              inputs[f'gidx_{cls}_{r}'][:, woff * 8:(woff + nwin) * 8])
                            gb = wpH.tile([128, mx, 128], F16, tag=f"gb{cls}")
                            mfh = mf[r][hop]
                            in_ap = mfh[0:LO_LIM, :] if cls == 'lo' else mfh[LO_LIM:NGP, :]
                            GW = 8
                            for w0 in range(0, nwin, GW):
                                sw = min(GW, nwin - w0)
                                nc.gpsimd.dma_gather(
                                    out_ap=gb[:, w0:w0 + sw, :], in_ap=in_ap,
                                    idxs_ap=it[:, w0 * 8:(w0 + sw) * 8],
                                    num_idxs=sw * 128, num_idxs_reg=sw * 128,
                                    elem_size=H)
                            ib = wpH.tile([128, mx, 128], F16, tag=f"ib{cls}")
                            nc.sync.dma_start(
                                ib[:, :nwin, :],
                                inputs[f'ind_{cls}_{r}'].ap()[:, woff * 128:(woff + nwin) * 128]
                                .rearrange("e (w d) -> e w d", w=nwin))
                            bufs[cls] = (gb, ib)
                        for t in tiles:
                            agg = psG.tile([128, 128], F32, tag="agg")
                            wins = ([('lo', lo_off[t] - lo_off[tiles[0]] + w) for w in range(lo_w[t])]
                                    + [('hi', hi_off[t] - hi_off[tiles[0]] + w) for w in range(hi_w[t])])
                            for wi, (cls, w) in enumerate(wins):
                                gb, ib = bufs[cls]
                                nc.tensor.matmul(agg[:], lhsT=ib[:, w, :], rhs=gb[:, w, :],
                                                 start=(wi == 0), stop=(wi == len(wins) - 1))
                            nc.vector.scalar_tensor_tensor(
                                fdst[:, t * 128:(t + 1) * 128], agg[:], ndv(r, t),
                                fsrc[:, t * 128:(t + 1) * 128],
                                op0=A.mult, op1=A.add)
                            if hop == 0:
                                m1 = wpH.tile([128, 128], F16, tag="m1")
                                nc.vector.tensor_scalar_mul(
                                    m1[:], fdst[:, t * 128:(t + 1) * 128], dv(r, t))
                                nc.sync.dma_start(ml[r][1][t * 128:(t + 1) * 128, :], m1[:])
                if hop == 0:
                    nc.gpsimd.collective_compute(
                        "AllGather", A.bypass,
                        replica_groups=[list(range(NCORES))],
                        ins=[ml[r][1].opt()], outs=[mf[r][1].opt()],
                    )

            def epilogue(r, psEt, psEw, wpE):
                # F1 = FB[r], F2 = FA[r] (node-major fp16); x_in reloaded from DRAM
                if True:
                    for (c0, cw) in CHUNKS:
                        nsub = cw // 128
                        X0c = wpE.tile([128, 512], F16, tag="X0c")
                        nc.sync.dma_start(X0c[:, :cw], xin_dram[r][:, c0:c0 + cw])
                        F1c = wpE.tile([128, 512], F16, tag="F1c")
                        F2c = wpE.tile([128, 512], F16, tag="F2c")
                        for si in range(nsub):
                            tr = psEt.tile([128, 128], F16, tag="tr")
                            nc.tensor.transpose(tr[:], FB[r][:, c0 + si * 128:c0 + (si + 1) * 128],
                                                wsl(wt, 'ident'))
                            nc.vector.tensor_copy(F1c[:, si * 128:(si + 1) * 128], tr[:])
                            tr2 = psEt.tile([128, 128], F16, tag="tr")
                            nc.tensor.transpose(tr2[:], FA[r][:, c0 + si * 128:c0 + (si + 1) * 128],
                                                wsl(wt, 'ident'))
                            nc.vector.tensor_copy(F2c[:, si * 128:(si + 1) * 128], tr2[:])
                        den_sb = wpE.tile([1, 512], F32, tag="den")
                        gsb = [wpE.tile([1, 512], F32, tag=f"gs{j}", name=f"gsb{j}")
                               for j in range(3)]
                        Bsrc = [X0c[:, :cw], F1c[:, :cw], F2c[:, :cw]]
                        for o in range(5):
                            pso = psEw.tile([128, 512], F32, tag="to")
                            js = [j for j in range(3) if CTRUE[o][j] != 0.0]
                            for ji, j in enumerate(js):
                                nc.tensor.matmul(pso[:, :cw], lhsT=wsl(wt, f'wf1_{r}_{o}_{j}'),
                                                 rhs=Bsrc[j], start=(ji == 0), stop=(ji == len(js) - 1))
                            To = wpE.tile([128, 512], F16, tag="To")
                            nc.scalar.activation(To[:, :cw], pso[:, :cw], AF.Tanh, bias=bias(f'bf1_{r}'))
                            psc = psEw.tile([1, 512], F32, tag="sc")
                            nc.tensor.matmul(psc[:, :cw], lhsT=wsl(wt, f'wf2_{r}'), rhs=To[:, :cw],
                                             start=True, stop=True)
                            eo = wpE.tile([1, 512], F16, tag="eo")
                            nc.scalar.activation(eo[:, :cw], psc[:, :cw], AF.Exp)
                            if o == 0:
                                nc.vector.tensor_copy(den_sb[:, :cw], eo[:, :cw])
                                for j in range(3):
                                    nc.vector.tensor_scalar_mul(gsb[j][:, :cw], eo[:, :cw],
                                                                float(CTRUE[o][j]))
                            else:
                                nc.vector.scalar_tensor_tensor(den_sb[:, :cw], eo[:, :cw], 0.0,
                                                               den_sb[:, :cw],
                                                               op0=A.bypass, op1=A.add)
                                for j in range(3):
                                    if CTRUE[o][j] != 0.0:
                                        nc.vector.scalar_tensor_tensor(
                                            gsb[j][:, :cw], eo[:, :cw], float(CTRUE[o][j]),
                                            gsb[j][:, :cw], op0=A.mult, op1=A.add)
                        rec = wpE.tile([1, 512], F32, tag="rec")
                        nc.vector.reciprocal(rec[:, :cw], den_sb[:, :cw])
                        res = wpE.tile([128, 512], F16, tag="res")
                        tmp = wpE.tile([128, 512], F16, tag="tmp")
                        for j in range(3):
                            gj = wpE.tile([1, 512], F16, tag="gj")
                            nc.vector.scalar_tensor_tensor(gj[:, :cw], rec[:, :cw], 0.0,
                                                           gsb[j][:, :cw], op0=A.bypass, op1=A.mult)
                            pbj = psEw.tile([128, 512], F32, tag="bc")
                            nc.tensor.matmul(pbj[:, :cw], lhsT=onesf16[:], rhs=gj[:, :cw],
                                             start=True, stop=True)
                            if j == 0:
                                nc.vector.scalar_tensor_tensor(res[:, :cw], Bsrc[j], 0.0, pbj[:, :cw],
                                                               op0=A.bypass, op1=A.mult)
                            else:
                                nc.vector.scalar_tensor_tensor(tmp[:, :cw], Bsrc[j], 0.0, pbj[:, :cw],
                                                               op0=A.bypass, op1=A.mult)
                                nc.vector.scalar_tensor_tensor(res[:, :cw], res[:, :cw], 0.0,
                                                               tmp[:, :cw], op0=A.bypass, op1=A.add)
                        ph = psEw.tile([128, 512], F32, tag="to")
                        nc.tensor.matmul(ph[:, :cw], lhsT=wsl(wt, f'lin5_{r}'), rhs=res[:, :cw],
                                         start=True, stop=True)
                        hall = wpE.tile([128, 512], F16, tag="hall")
                        nc.scalar.activation(hall[:, :cw], ph[:, :cw], AF.Lrelu,
                                             bias=bias(f'b5_{r}'), alpha=0.01)
                        po = psEw.tile([2, 512], F32, tag="po")
                        nc.tensor.matmul(po[:, :cw], lhsT=wsl(wt, f'lin6_{r}'), rhs=hall[:, :cw],
                                         start=True, stop=True)
                        oo = wpE.tile([2, 512], F32, tag="oo")
                        nc.scalar.copy(oo[:, :cw], po[:, :cw])
                        nc.sync.dma_start(out_parts[r, :, c0:c0 + cw], oo[:, :cw])

            # software pipeline: hop0 x3 (each retriggers its AG for hop1),
            # then hop1 + epilogue per relation
            import os as _os
            _KPART = _os.environ.get("KPART", "ALL")
            with (tc.tile_pool(name="psG", bufs=2, space="PSUM") as psG,
                  tc.tile_pool(name="wpH", bufs=3) as wpH,
                  tc.tile_pool(name="psEt", bufs=2, space="PSUM") as psEt,
                  tc.tile_pool(name="psEw", bufs=1, space="PSUM") as psEw,
                  tc.tile_pool(name="wpE", bufs=2) as wpE):
                if _KPART != "A":
                    for r in range(R if _KPART in ("ALL", "H1") else 1):
                        process_hop(r, 0, psG, wpH)
                if _KPART in ("ALL", "H1"):
                    for r in range(R):
                        process_hop(r, 1, psG, wpH)
                        if _KPART == "ALL":
                            epilogue(r, psEt, psEw, wpE)

    nc.compile()


def kernel(**inp):
    meta, cores = _prep(inp)
    nc = bacc.Bacc("TRN2", target_bir_lowering=False, debug=False, num_devices=NCORES)
    _build(nc, meta)
    res = run_bass_kernel_spmd(nc, [dict(c) for c in cores], core_ids=list(range(NCORES)))
    out = np.zeros((N, C), np.float32)
    b6 = np.asarray(inp['b_lin6'], np.float32)
    for c in range(NCORES):
        parts = res.results[c]["out_parts"]
        out[c * NL:(c + 1) * NL] = parts.sum(axis=0).T[:NL] + b6[None, :]
    return out


if __name__ == "__main__":
    import reference
    inputs = {k: np.asarray(v) for k, v in reference.setup_inputs().items()}
    got = kernel(**inputs)
    exp = np.asarray(reference.reference(**inputs))
    err = np.abs(got - exp).max()
    rel = err / max(np.abs(exp).max(), 1e-9)
    print("abs err:", err, "rel err:", rel)


# revision 38
# speedup vs baseline: 1.9383x; 1.5008x over previous
"""BWGNN_Hetero Trainium2 kernel: 8-core SPMD, node-sharded graph/data parallel.

v2: fp16 tensor-engine path (merged M=128 LSTM gate matmuls, fp16 weights/
activations, fp16 feature upload), software-pipelined Phase B (all three
relations' hop-0 AllGathers issued at the end of Phase A; hops interleaved
across relations so collectives and tensor epilogues hide under the gpsimd
dma_gather stream), single fused attention epilogue pass, and one dma_gather
call per (tile-group, class).

Algorithmic reduction: the 5 beta-wavelet filters are linear combinations of
{f0, f1=L f0, f2=L f1} (L = normalized Laplacian), so each relation needs
only 2 sparse hops.
"""
import sys
sys.path.insert(0, '/opt/trn_rl_repo')
sys.path.insert(0, '/root/problem')

import numpy as np

import concourse.bacc as bacc
import concourse.bass as bass
import concourse.mybir as mybir
import concourse.tile as tile
from concourse.bass_utils import run_bass_kernel_spmd

F32 = mybir.dt.float32
F16 = mybir.dt.float16
I16 = mybir.dt.int16
A = mybir.AluOpType
AF = mybir.ActivationFunctionType

NCORES = 8
N, E, R, T = 50000, 800000, 3, 16
IV, IS, IP, H, C = 64, 64, 32, 128, 2
NL = N // NCORES            # 6250
NT = 49                     # dst tiles per core
NLP = NT * 128              # 6272 padded local nodes
NGP = NLP * NCORES          # 50176 padded global rows in AllGather output
LO_LIM = 32768              # int16 gather index limit
GS = 2                      # dst tiles per gather group
NSWQ = 2                    # SWDGE queues for gather descriptor rings
CHUNKS = [(i * 512, 512) for i in range(12)] + [(6144, 128)]

CTRUE = [[0.8, -0.5, 0.0],
         [3.0, -3.0, 0.75],
         [0.0, 3.0, -1.5],
         [0.0, 0.0, 0.75],
         [-0.2, 0.5, 0.0]]


def _wrap_idx(idx):
    """[n] int16 -> [128, ceil(n/16)] wrapped (i -> [i%16, i//16]) + replicated x8."""
    n = len(idx)
    L = max(1, (n + 15) // 16)
    a = np.zeros((16, L), np.int16)
    for p in range(16):
        vals = idx[p::16]
        a[p, :len(vals)] = vals
    return np.tile(a, (8, 1))


class WPack:
    def __init__(self, dtype):
        self.dtype = dtype
        self.cols = []
        self.off = 0
        self.slots = {}

    def add(self, name, mat, row0=0):
        mat = np.asarray(mat, self.dtype)
        k, m = mat.shape
        assert row0 + k <= 128
        buf = np.zeros((128, m), self.dtype)
        buf[row0:row0 + k] = mat
        self.cols.append(buf)
        self.slots[name] = (row0, k, self.off, m)
        self.off += m

    def image(self):
        return np.concatenate(self.cols, axis=1)


def _prep(inp):
    g = {k: np.asarray(v) for k, v in inp.items()}
    wp = WPack(np.float16)

    # torch gate-row offsets: i=0, f=64, g=128, o=192
    # XHv rows: [h_v (0:64); x_v (64:128)];  XHs rows: [x_s (0:64); h_s (64:128)]
    def lv_pair(g1, g2):
        m = np.zeros((128, 128), np.float32)
        for ci, ro in enumerate((g1, g2)):
            m[0:64, ci * 64:(ci + 1) * 64] = g['Whh_v'][ro:ro + 64, :].T
            m[64:128, ci * 64:(ci + 1) * 64] = g['Wih_v'][ro:ro + 64, :].T
        return m

    def ls_pair(g1, g2):
        m = np.zeros((128, 128), np.float32)
        for ci, ro in enumerate((g1, g2)):
            m[0:64, ci * 64:(ci + 1) * 64] = g['Wih_s'][ro:ro + 64, :].T
            m[64:128, ci * 64:(ci + 1) * 64] = g['Whh_s'][ro:ro + 64, :].T
        return m

    wp.add('lv_if', lv_pair(0, 64))
    wp.add('lv_og', lv_pair(192, 128))
    wp.add('ls_if', ls_pair(0, 64))
    wp.add('ls_og', ls_pair(192, 128))
    wp.add('lin', g['W_lin'].T)                       # rows 0:64 (rhs = h_v at base 0)
    wp.add('lin1', g['W_lin1'].T, row0=64)            # rows 64:128 (rhs = h_s at base 64)
    wp.add('pers', g['W_pers'].T)
    wp.add('lin2a', g['W_lin2'][:, 0:128].T)
    wp.add('lin2b', g['W_lin2'][:, 128:256].T)
    wp.add('lin3a', g['W_lin3'][:, 0:128].T)
    wp.add('lin3b', g['W_lin3'][:, 128:256].T)
    wp.add('lin4a', g['W_lin4'][:, 0:128].T)
    wp.add('lin4bd', (g['W_lin4'][:, 128:256] + g['W_lin4'][:, 384:512]).T)
    wp.add('lin4c', g['W_lin4'][:, 256:384].T)
    for r in range(R):
        for o in range(5):
            for j in range(3):
                if CTRUE[o][j] != 0.0:
                    wp.add(f'wf1_{r}_{o}_{j}', (CTRUE[o][j] * g['Wf1'][r]).T)
        wp.add(f'wf2_{r}', g['Wf2'][r][:, None])
        wp.add(f'lin5_{r}', g['W_lin5'][r].T)
    for k in range(6):
        wp.add(f'lin6_{k}', g['W_lin6'][:, k * 128:(k + 1) * 128].T)
    wp.add('ident', np.eye(128, dtype=np.float32))
    wp.add('one11', np.ones((1, 1), np.float32))
    for o in range(5):
        for j in range(3):
            wp.add(f'c_{o}_{j}', np.array([[CTRUE[o][j]]], np.float32))
    wimg = wp.image()

    bcols, blist = {}, []

    def addb(name, vec):
        bcols[name] = len(blist)
        v = np.zeros((128, 1), np.float32)
        v[:len(vec), 0] = np.asarray(vec, np.float32).ravel()
        blist.append(v)

    bv = g['bih_v'] + g['bhh_v']
    bs = g['bih_s'] + g['bhh_s']
    addb('b_if_v', np.concatenate([bv[0:64], bv[64:128]]))     # [i_v; f_v]
    addb('b_o_v', bv[192:256])
    addb('b_g_v', bv[128:192])
    addb('b_i_s', bs[0:64])
    addb('b_f_s', bs[64:128])
    addb('b_o_s', bs[192:256])
    addb('b_g_s', bs[128:192])
    addb('b_lin', g['b_lin'])
    addb('b_lin1', g['b_lin1'])
    addb('b_pers', g['b_pers'])
    addb('b2', g['b_lin2'])
    addb('b3', g['b_lin3'])
    addb('b4', g['b_lin4'])
    for r in range(R):
        addb(f'bf1_{r}', g['bf1'][r])
        addb(f'b5_{r}', g['b_lin5'][r])
    bimg = np.concatenate(blist, axis=1)

    src = np.asarray(g['src'], np.int64)
    dst = np.asarray(g['dst'], np.int64)
    gsrc_all = (src // NL) * NLP + (src % NL)

    percore = [dict() for _ in range(NCORES)]
    relmeta = []
    for r in range(R):
        deg = np.bincount(dst[r], minlength=N).astype(np.float32)
        dinv = np.clip(deg, 1.0, None) ** -0.5

        # bucket edges: (core, tile, class)
        per = []
        for c in range(NCORES):
            m = (dst[r] // NL) == c
            sc = gsrc_all[r][m]
            dl = dst[r][m] - c * NL
            tl, col = dl // 128, dl % 128
            tiles = []
            for t in range(NT):
                mt = tl == t
                st_, ct_ = sc[mt], col[mt]
                lo = st_ < LO_LIM
                tiles.append((st_[lo], ct_[lo], st_[~lo] - LO_LIM, ct_[~lo]))
            per.append(tiles)
        # common (max-over-cores) window counts
        lo_w = [max(1, max((len(per[c][t][0]) + 127) // 128 for c in range(NCORES)))
                for t in range(NT)]
        hi_w = [max(1, max((len(per[c][t][2]) + 127) // 128 for c in range(NCORES)))
                for t in range(NT)]
        relmeta.append({'lo_w': lo_w, 'hi_w': hi_w})
        for c in range(NCORES):
            li_s, lc_s, hi_s, hc_s = [], [], [], []
            for t in range(NT):
                li, lc, hi, hc = per[c][t]
                lp = np.zeros(lo_w[t] * 128, np.int64); lp[:len(li)] = li
                lcp = np.full(lo_w[t] * 128, -1, np.int64); lcp[:len(lc)] = lc
                hp = np.zeros(hi_w[t] * 128, np.int64); hp[:len(hi)] = hi
                hcp = np.full(hi_w[t] * 128, -1, np.int64); hcp[:len(hc)] = hc
                li_s.append(lp); lc_s.append(lcp); hi_s.append(hp); hc_s.append(hcp)
            li_s = np.concatenate(li_s); lc_s = np.concatenate(lc_s)
            hi_s = np.concatenate(hi_s); hc_s = np.concatenate(hc_s)

            def mkind(colarr):
                # wrapped layout [e, w*128 + c]: partition = edge-in-window, so the
                # device DMA is a contiguous per-partition copy (no 256B scatter)
                W = len(colarr) // 128
                ind = np.zeros((128, W * 128), np.float16)
                valid = colarr >= 0
                pos = np.nonzero(valid)[0]
                w, e = pos // 128, pos % 128
                ind[e, w * 128 + colarr[valid]] = 1.0
                return ind
            pc = percore[c]
            pc[f'gidx_lo_{r}'] = _wrap_idx(li_s.astype(np.int16))
            pc[f'gidx_hi_{r}'] = _wrap_idx(hi_s.astype(np.int16))
            pc[f'ind_lo_{r}'] = mkind(lc_s)
            pc[f'ind_hi_{r}'] = mkind(hc_s)
            dp = pc.setdefault('_dinv', np.zeros((128, 2 * R * NT), np.float32))
            dvl = np.ones(NLP, np.float32)
            dvl[:NL] = dinv[c * NL:(c + 1) * NL]
            dp[:, r * NT:(r + 1) * NT] = dvl.reshape(NT, 128).T
            dp[:, R * NT + r * NT:R * NT + (r + 1) * NT] = -dvl.reshape(NT, 128).T

    voc = np.asarray(g['voc_features'], np.float16)
    sms = np.asarray(g['sms_features'], np.float16)
    pers = np.asarray(g['personal_feature'], np.float16)
    cores = []
    for c in range(NCORES):
        pc = percore[c]
        sl = slice(c * NL, (c + 1) * NL)
        vt = np.zeros((T, IV, NLP), np.float16)
        st_ = np.zeros((T, IS, NLP), np.float16)
        vt[:, :, :NL] = voc[sl].transpose(1, 2, 0)
        st_[:, :, :NL] = sms[sl].transpose(1, 2, 0)
        pt = np.zeros((IP, NLP), np.float16)
        pt[:, :NL] = pers[sl].T
        pc['voc'] = vt
        pc['sms'] = st_
        pc['pers'] = pt
        pc['wpack'] = wimg
        pc['bpack'] = bimg
        pc['dpack'] = pc.pop('_dinv')
        cores.append(pc)
    meta = {
        'wp': wp.slots, 'bcols': bcols, 'rel': relmeta,
        'shapes': {k: v.shape for k, v in cores[0].items()},
        'dtypes': {k: v.dtype for k, v in cores[0].items()},
    }
    return meta, cores


def _build(nc, meta):
    sh, dt = meta['shapes'], meta['dtypes']
    WP, BC = meta['wp'], meta['bcols']
    inputs = {k: nc.dram_tensor(k, list(sh[k]), mybir.dt.from_np(np.dtype(dt[k])),
                                kind="ExternalInput") for k in sh}
    out_parts = nc.dram_tensor("out_parts", [4, 2, NLP], F32, kind="ExternalOutput")

    def wsl(wt, name):
        r0, k, off, m = WP[name]
        return wt[r0:r0 + k, off:off + m]

    with tile.TileContext(nc) as tc:
        with (
            tc.tile_pool(name="const", bufs=1) as cpool,
            tc.tile_pool(name="persist", bufs=1) as spool,
            tc.tile_pool(name="dram", bufs=1, space="DRAM") as dpool,
        ):
            wt = cpool.tile([128, sh['wpack'][1]], F16)
            nc.sync.dma_start(wt[:], inputs['wpack'][:])
            bt = cpool.tile([128, sh['bpack'][1]], F32)
            nc.sync.dma_start(bt[:], inputs['bpack'][:])
            dpt = cpool.tile([128, 2 * R * NT], F32)
            nc.sync.dma_start(dpt[:], inputs['dpack'][:])
            onesf16 = cpool.tile([1, 128], F16)
            nc.vector.memset(onesf16[:], 1.0)

            def bias(name, p0=0, pn=128):
                return bt[p0:p0 + pn, BC[name]:BC[name] + 1]

            # persistent fp16 node-major filter states: FA_r (f0, later f2), FB_r (f1)
            FA = [spool.tile([128, NLP], F16, name=f"FA{r}") for r in range(R)]
            FB = [spool.tile([128, NLP], F16, name=f"FB{r}") for r in range(R)]
            xin_dram = [dpool.tile([128, NLP], F16, tag=f"xin{r}", name=f"xind{r}")
                        for r in range(R)]
            ml = [[dpool.tile([NLP, H], F16, tag=f"ml{r}h{h}", name=f"mld{r}h{h}")
                   for h in range(2)] for r in range(R)]
            mf = [[dpool.tile([NGP, H], F16, tag=f"mf{r}h{h}", name=f"mfd{r}h{h}",
                              addr_space="Shared") for h in range(2)] for r in range(R)]

            def dv(r, t):
                return dpt[:, r * NT + t:r * NT + t + 1]

            def ndv(r, t):
                return dpt[:, R * NT + r * NT + t:R * NT + r * NT + t + 1]

            # =============== Phase A: fused LSTMs ===============
            with (tc.tile_pool(name="stA", bufs=1) as stA,
                  tc.tile_pool(name="wpA", bufs=2) as wpA):
                XHv = stA.tile([128, NLP], F16)
                XHs = stA.tile([128, NLP], F16)
                Cst = stA.tile([128, NLP], F32)   # rows 64:128 = c_v, rows 0:64 = c_s
                nc.vector.memset(XHv[0:64, :], 0.0)
                nc.vector.memset(XHs[64:128, :], 0.0)
                nc.vector.memset(Cst[:], 0.0)
                with tc.tile_pool(name="psA", bufs=2, space="PSUM") as psA:
                    for t in range(T):
                        nc.sync.dma_start(XHv[64:128, :], inputs['voc'][t])
                        nc.sync.dma_start(XHs[0:64, :], inputs['sms'][t])
                        for (c0, cw) in CHUNKS:
                            PA = psA.tile([128, 512], F32, tag="pa")  # [i_v; f_v]
                            PB = psA.tile([128, 512], F32, tag="pb")  # [o_v; g_v]
                            PC = psA.tile([128, 512], F32, tag="pc")  # [i_s; f_s]
                            PD = psA.tile([128, 512], F32, tag="pd")  # [o_s; g_s]
                            nc.tensor.matmul(PA[:, :cw], lhsT=wsl(wt, 'lv_if'),
                                             rhs=XHv[:, c0:c0 + cw], start=True, stop=True)
                            nc.tensor.matmul(PB[:, :cw], lhsT=wsl(wt, 'lv_og'),
                                             rhs=XHv[:, c0:c0 + cw], start=True, stop=True)
                            nc.tensor.matmul(PC[:, :cw], lhsT=wsl(wt, 'ls_if'),
                                             rhs=XHs[:, c0:c0 + cw], start=True, stop=True)
                            nc.tensor.matmul(PD[:, :cw], lhsT=wsl(wt, 'ls_og'),
                                             rhs=XHs[:, c0:c0 + cw], start=True, stop=True)
                            # v-source: SA = sigmoid([i_v; f_v]) full 128
                            SA = wpA.tile([128, 512], F16, tag="SA")
                            nc.scalar.activation(SA[:, :cw], PA[:, :cw], AF.Sigmoid,
                                                 bias=bias('b_if_v'))
                            Sov = wpA.tile([64, 512], F16, tag="Sov")
                            nc.scalar.activation(Sov[:, :cw], PB[0:64, :cw], AF.Sigmoid,
                                                 bias=bias('b_o_v', 0, 64))
                            Tgv = wpA.tile([64, 512], F16, tag="Tgv")
                            nc.scalar.activation(Tgv[:, :cw], PB[64:128, :cw], AF.Tanh,
                                                 bias=bias('b_g_v', 0, 64))
                            # s-source split sigmoids (base-0 outputs)
                            Sis = wpA.tile([64, 512], F16, tag="Sis")
                            nc.scalar.activation(Sis[:, :cw], PC[0:64, :cw], AF.Sigmoid,
                                                 bias=bias('b_i_s', 0, 64))
                            Sfs = wpA.tile([64, 512], F16, tag="Sfs")
                            nc.scalar.activation(Sfs[:, :cw], PC[64:128, :cw], AF.Sigmoid,
                                                 bias=bias('b_f_s', 0, 64))
                            Sos = wpA.tile([64, 512], F16, tag="Sos")
                            nc.scalar.activation(Sos[:, :cw], PD[0:64, :cw], AF.Sigmoid,
                                                 bias=bias('b_o_s', 0, 64))
                            Tgs = wpA.tile([64, 512], F16, tag="Tgs")
                            nc.scalar.activation(Tgs[:, :cw], PD[64:128, :cw], AF.Tanh,
                                                 bias=bias('b_g_s', 0, 64))
                            # c_v (rows 64:128 of Cst): u = f_v*c_v ; v = i_v*g_v
                            uv = wpA.tile([64, 512], F32, tag="uv")
                            nc.vector.scalar_tensor_tensor(uv[:, :cw], SA[64:128, :cw], 0.0,
                                                           Cst[64:128, c0:c0 + cw],
                                                           op0=A.bypass, op1=A.mult)
                            vv = wpA.tile([64, 512], F32, tag="vv")
                            nc.vector.scalar_tensor_tensor(vv[:, :cw], SA[0:64, :cw], 0.0,
                                                           Tgv[:, :cw], op0=A.bypass, op1=A.mult)
                            nc.vector.scalar_tensor_tensor(Cst[64:128, c0:c0 + cw], uv[:, :cw],
                                                           0.0, vv[:, :cw],
                                                           op0=A.bypass, op1=A.add)
                            tcv = wpA.tile([64, 512], F32, tag="tcv")
                            nc.scalar.activation(tcv[:, :cw], Cst[64:128, c0:c0 + cw], AF.Tanh)
                            nc.vector.scalar_tensor_tensor(XHv[0:64, c0:c0 + cw], Sov[:, :cw],
                                                           0.0, tcv[:, :cw],
                                                           op0=A.bypass, op1=A.mult)
                            # c_s (rows 0:64 of Cst)
                            us = wpA.tile([64, 512], F32, tag="us")
                            nc.vector.scalar_tensor_tensor(us[:, :cw], Sfs[:, :cw], 0.0,
                                                           Cst[0:64, c0:c0 + cw],
                                                           op0=A.bypass, op1=A.mult)
                            vs = wpA.tile([64, 512], F32, tag="vs")
                            nc.vector.scalar_tensor_tensor(vs[:, :cw], Sis[:, :cw], 0.0,
                                                           Tgs[:, :cw], op0=A.bypass, op1=A.mult)
                            nc.vector.scalar_tensor_tensor(Cst[0:64, c0:c0 + cw], us[:, :cw],
                                                           0.0, vs[:, :cw],
                                                           op0=A.bypass, op1=A.add)
                            tcs = wpA.tile([64, 512], F32, tag="tcs")
                            nc.scalar.activation(tcs[:, :cw], Cst[0:64, c0:c0 + cw], AF.Tanh)
                            nc.vector.scalar_tensor_tensor(XHs[64:128, c0:c0 + cw], Sos[:, :cw],
                                                           0.0, tcs[:, :cw],
                                                           op0=A.bypass, op1=A.mult)

                # ---- Phase A epilogue: head + inline transposes/messages
                PT = stA.tile([32, NLP], F16)
                nc.sync.dma_start(PT[:], inputs['pers'][:])
                with (tc.tile_pool(name="psB", bufs=2, space="PSUM") as psB,
                      tc.tile_pool(name="psBs", bufs=2, space="PSUM") as psBs):
                    for (c0, cw) in CHUNKS:
                        pxa = psB.tile([128, 512], F32, tag="pa")
                        nc.tensor.matmul(pxa[:, :cw], lhsT=wsl(wt, 'lin'),
                                         rhs=XHv[0:64, c0:c0 + cw], start=True, stop=True)
                        pxp = psB.tile([128, 512], F32, tag="pb")
                        nc.tensor.matmul(pxp[:, :cw], lhsT=wsl(wt, 'pers'),
                                         rhs=PT[:, c0:c0 + cw], start=True, stop=True)
                        pxs = psB.tile([128, 512], F32, tag="pc")
                        nc.tensor.matmul(pxs[:, :cw], lhsT=wsl(wt, 'lin1'),
                                         rhs=XHs[64:128, c0:c0 + cw], start=True, stop=True)
                        XA = wpA.tile([128, 512], F16, tag="XA")
                        XP = wpA.tile([128, 512], F16, tag="XP")
                        XS = wpA.tile([128, 512], F16, tag="XS")
                        nc.scalar.activation(XA[:, :cw], pxa[:, :cw], AF.Lrelu, bias=bias('b_lin'), alpha=0.01)
                        nc.scalar.activation(XP[:, :cw], pxp[:, :cw], AF.Lrelu, bias=bias('b_pers'), alpha=0.01)
                        nc.scalar.activation(XS[:, :cw], pxs[:, :cw], AF.Lrelu, bias=bias('b_lin1'), alpha=0.01)
                        p0 = psB.tile([128, 512], F32, tag="pa")
                        nc.tensor.matmul(p0[:, :cw], lhsT=wsl(wt, 'lin2a'), rhs=XA[:, :cw], start=True, stop=False)
                        nc.tensor.matmul(p0[:, :cw], lhsT=wsl(wt, 'lin2b'), rhs=XP[:, :cw], start=False, stop=True)
                        p1 = psB.tile([128, 512], F32, tag="pb")
                        nc.tensor.matmul(p1[:, :cw], lhsT=wsl(wt, 'lin3a'), rhs=XS[:, :cw], start=True, stop=False)
                        nc.tensor.matmul(p1[:, :cw], lhsT=wsl(wt, 'lin3b'), rhs=XP[:, :cw], start=False, stop=True)
                        p2 = psB.tile([128, 512], F32, tag="pc")
                        nc.tensor.matmul(p2[:, :cw], lhsT=wsl(wt, 'lin4a'), rhs=XA[:, :cw], start=True, stop=False)
                        nc.tensor.matmul(p2[:, :cw], lhsT=wsl(wt, 'lin4bd'), rhs=XP[:, :cw], start=False, stop=False)
                        nc.tensor.matmul(p2[:, :cw], lhsT=wsl(wt, 'lin4c'), rhs=XS[:, :cw], start=False, stop=True)
                        Xc = [wpA.tile([128, 512], F16, tag=f"X{r}c", name=f"Xc{r}") for r in range(R)]
                        nc.scalar.activation(Xc[0][:, :cw], p0[:, :cw], AF.Lrelu, bias=bias('b2'), alpha=0.01)
                        nc.scalar.activation(Xc[1][:, :cw], p1[:, :cw], AF.Lrelu, bias=bias('b3'), alpha=0.01)
                        nc.scalar.activation(Xc[2][:, :cw], p2[:, :cw], AF.Lrelu, bias=bias('b4'), alpha=0.01)
                        for r in range(R):
                            nc.sync.dma_start(xin_dram[r][:, c0:c0 + cw], Xc[r][:, :cw])
                        p6 = psBs.tile([2, 512], F32, tag="p6")
                        nc.tensor.matmul(p6[:, :cw], lhsT=wsl(wt, 'lin6_3'), rhs=Xc[0][:, :cw],
                                         start=True, stop=False)
                        nc.tensor.matmul(p6[:, :cw], lhsT=wsl(wt, 'lin6_4'), rhs=Xc[1][:, :cw],
                                         start=False, stop=False)
                        nc.tensor.matmul(p6[:, :cw], lhsT=wsl(wt, 'lin6_5'), rhs=Xc[2][:, :cw],
                                         start=False, stop=True)
                        o6 = wpA.tile([2, 512], F32, tag="o6")
                        nc.scalar.copy(o6[:, :cw], p6[:, :cw])
                        nc.sync.dma_start(out_parts[3, :, c0:c0 + cw], o6[:, :cw])

            # transposes -> F0 (node-major) + hop-0 messages + hop-0 AllGathers
            with (tc.tile_pool(name="psT", bufs=2, space="PSUM") as psT,
                  tc.tile_pool(name="wpT", bufs=2) as wpT):
                for r in range(R):
                    for t in range(NT):
                        xt = wpT.tile([128, 128], F16, tag="xt")
                        nc.sync.dma_start(xt[:], xin_dram[r][:, t * 128:(t + 1) * 128])
                        tr = psT.tile([128, 128], F16, tag="tr")
                        nc.tensor.transpose(tr[:], xt[:], wsl(wt, 'ident'))
                        nc.vector.tensor_copy(FA[r][:, t * 128:(t + 1) * 128], tr[:])
                        m1 = wpT.tile([128, 128], F16, tag="m1")
                        nc.vector.tensor_scalar_mul(m1[:], tr[:], dv(r, t))
                        nc.sync.dma_start(ml[r][0][t * 128:(t + 1) * 128, :], m1[:])
                    nc.gpsimd.collective_compute(
                        "AllGather", A.bypass,
                        replica_groups=[list(range(NCORES))],
                        ins=[ml[r][0].opt()], outs=[mf[r][0].opt()],
                    )

            # =============== Phase B ===============
            MAXLO = max(max(sum(meta['rel'][r]['lo_w'][t0:t0 + GS])
                            for t0 in range(0, NT, GS)) for r in range(R))
            MAXHI = max(max(sum(meta['rel'][r]['hi_w'][t0:t0 + GS])
                            for t0 in range(0, NT, GS)) for r in range(R))

            gq = [0]            # round-robin gather queue selector

            def process_hop(r, hop, psG, wpH):
                relm = meta['rel'][r]
                lo_w, hi_w = relm['lo_w'], relm['hi_w']
                lo_off, hi_off = [0], [0]
                for t in range(NT):
                    lo_off.append(lo_off[-1] + lo_w[t])
                    hi_off.append(hi_off[-1] + hi_w[t])
                fsrc = FA[r] if hop == 0 else FB[r]
                fdst = FB[r] if hop == 0 else FA[r]
                if True:
                    for t0g in range(0, NT, GS):
                        tiles = list(range(t0g, min(t0g + GS, NT)))
                        nlo = sum(lo_w[t] for t in tiles)
                        nhi = sum(hi_w[t] for t in tiles)
                        bufs = {}
                        for cls, nwin, mx, woff in (('lo', nlo, MAXLO, lo_off[tiles[0]]),
                                                    ('hi', nhi, MAXHI, hi_off[tiles[0]])):
                            it = wpH.tile([128, mx * 8], I16, tag=f"idx{cls}")
                            nc.sync.dma_start(it[:, :nwin * 8],
                                              inputs[f'gidx_{cls}_{r}'][:, woff * 8:(woff + nwin) * 8])
                            gb = wpH.tile([128, mx, 128], F16, tag=f"gb{cls}")
                            mfh = mf[r][hop]
                            in_ap = mfh[0:LO_LIM, :] if cls == 'lo' else mfh[LO_LIM:NGP, :]
                            GW = 8
                            for w0 in range(0, nwin, GW):
                                sw = min(GW, nwin - w0)
                                q = gq[0]
                                gq[0] = (q + 1) % NSWQ
                                nc.gpsimd.dma_gather(
                                    out_ap=gb[:, w0:w0 + sw, :], in_ap=in_ap,
                                    idxs_ap=it[:, w0 * 8:(w0 + sw) * 8],
                                    num_idxs=sw * 128, num_idxs_reg=sw * 128,
                                    elem_size=H, queue_num=q)
                            ib = wpH.tile([128, mx, 128], F16, tag=f"ib{cls}")
                            nc.sync.dma_start(
                                ib[:, :nwin, :],
                                inputs[f'ind_{cls}_{r}'].ap()[:, woff * 128:(woff + nwin) * 128]
                                .rearrange("e (w d) -> e w d", w=nwin))
                            bufs[cls] = (gb, ib)
                        for t in tiles:
                            agg = psG.tile([128, 128], F32, tag="agg")
                            wins = ([('lo', lo_off[t] - lo_off[tiles[0]] + w) for w in range(lo_w[t])]
                                    + [('hi', hi_off[t] - hi_off[tiles[0]] + w) for w in range(hi_w[t])])
                            for wi, (cls, w) in enumerate(wins):
                                gb, ib = bufs[cls]
                                nc.tensor.matmul(agg[:], lhsT=ib[:, w, :], rhs=gb[:, w, :],
                                                 start=(wi == 0), stop=(wi == len(wins) - 1))
                            nc.vector.scalar_tensor_tensor(
                                fdst[:, t * 128:(t + 1) * 128], agg[:], ndv(r, t),
                                fsrc[:, t * 128:(t + 1) * 128],
                                op0=A.mult, op1=A.add)
                            if hop == 0:
                                m1 = wpH.tile([128, 128], F16, tag="m1")
                                # scalar engine is idle in Phase B; DVE tensor_scalar
                                # stalls ~4us here (SWDGE ring contention)
                                nc.scalar.activation(
                                    m1[:], fdst[:, t * 128:(t + 1) * 128], AF.Copy,
                                    scale=dv(r, t))
                                nc.sync.dma_start(ml[r][1][t * 128:(t + 1) * 128, :], m1[:])
                if hop == 0:
                    nc.gpsimd.collective_compute(
                        "AllGather", A.bypass,
                        replica_groups=[list(range(NCORES))],
                        ins=[ml[r][1].opt()], outs=[mf[r][1].opt()],
                    )

            def epilogue(r, psEt, psEw, wpE):
                # F1 = FB[r], F2 = FA[r] (node-major fp16); x_in reloaded from DRAM
                if True:
                    for (c0, cw) in CHUNKS:
                        nsub = cw // 128
                        X0c = wpE.tile([128, 512], F16, tag="X0c")
                        nc.sync.dma_start(X0c[:, :cw], xin_dram[r][:, c0:c0 + cw])
                        F1c = wpE.tile([128, 512], F16, tag="F1c")
                        F2c = wpE.tile([128, 512], F16, tag="F2c")
                        for si in range(nsub):
                            tr = psEt.tile([128, 128], F16, tag="tr")
                            nc.tensor.transpose(tr[:], FB[r][:, c0 + si * 128:c0 + (si + 1) * 128],
                                                wsl(wt, 'ident'))
                            nc.vector.tensor_copy(F1c[:, si * 128:(si + 1) * 128], tr[:])
                            tr2 = psEt.tile([128, 128], F16, tag="tr")
                            nc.tensor.transpose(tr2[:], FA[r][:, c0 + si * 128:c0 + (si + 1) * 128],
                                                wsl(wt, 'ident'))
                            nc.vector.tensor_copy(F2c[:, si * 128:(si + 1) * 128], tr2[:])
                        den_sb = wpE.tile([1, 512], F32, tag="den")
                        gsb = [wpE.tile([1, 512], F32, tag=f"gs{j}", name=f"gsb{j}")
                               for j in range(3)]
                        Bsrc = [X0c[:, :cw], F1c[:, :cw], F2c[:, :cw]]
                        for o in range(5):
                            pso = psEw.tile([128, 512], F32, tag="to")
                            js = [j for j in range(3) if CTRUE[o][j] != 0.0]
                            for ji, j in enumerate(js):
                                nc.tensor.matmul(pso[:, :cw], lhsT=wsl(wt, f'wf1_{r}_{o}_{j}'),
                                                 rhs=Bsrc[j], start=(ji == 0), stop=(ji == len(js) - 1))
                            To = wpE.tile([128, 512], F16, tag="To")
                            nc.scalar.activation(To[:, :cw], pso[:, :cw], AF.Tanh, bias=bias(f'bf1_{r}'))
                            psc = psEw.tile([1, 512], F32, tag="sc")
                            nc.tensor.matmul(psc[:, :cw], lhsT=wsl(wt, f'wf2_{r}'), rhs=To[:, :cw],
                                             start=True, stop=True)
                            eo = wpE.tile([1, 512], F16, tag="eo")
                            nc.scalar.activation(eo[:, :cw], psc[:, :cw], AF.Exp)
                            if o == 0:
                                nc.vector.tensor_copy(den_sb[:, :cw], eo[:, :cw])
                                for j in range(3):
                                    nc.vector.tensor_scalar_mul(gsb[j][:, :cw], eo[:, :cw],
                                                                float(CTRUE[o][j]))
                            else:
                                nc.vector.scalar_tensor_tensor(den_sb[:, :cw], eo[:, :cw], 0.0,
                                                               den_sb[:, :cw],
                                                               op0=A.bypass, op1=A.add)
                                for j in range(3):
                                    if CTRUE[o][j] != 0.0:
                                        nc.vector.scalar_tensor_tensor(
                                            gsb[j][:, :cw], eo[:, :cw], float(CTRUE[o][j]),
                                            gsb[j][:, :cw], op0=A.mult, op1=A.add)
                        rec = wpE.tile([1, 512], F32, tag="rec")
                        nc.vector.reciprocal(rec[:, :cw], den_sb[:, :cw])
                        res = wpE.tile([128, 512], F16, tag="res")
                        tmp = wpE.tile([128, 512], F16, tag="tmp")
                        for j in range(3):
                            gj = wpE.tile([1, 512], F16, tag="gj")
                            nc.vector.scalar_tensor_tensor(gj[:, :cw], rec[:, :cw], 0.0,
                                                           gsb[j][:, :cw], op0=A.bypass, op1=A.mult)
                            pbj = psEw.tile([128, 512], F32, tag="bc")
                            nc.tensor.matmul(pbj[:, :cw], lhsT=onesf16[:], rhs=gj[:, :cw],
                                             start=True, stop=True)
                            if j == 0:
                                nc.vector.scalar_tensor_tensor(res[:, :cw], Bsrc[j], 0.0, pbj[:, :cw],
                                                               op0=A.bypass, op1=A.mult)
                            else:
                                nc.vector.scalar_tensor_tensor(tmp[:, :cw], Bsrc[j], 0.0, pbj[:, :cw],
                                                               op0=A.bypass, op1=A.mult)
                                nc.vector.scalar_tensor_tensor(res[:, :cw], res[:, :cw], 0.0,
                                                               tmp[:, :cw], op0=A.bypass, op1=A.add)
                        ph = psEw.tile([128, 512], F32, tag="to")
                        nc.tensor.matmul(ph[:, :cw], lhsT=wsl(wt, f'lin5_{r}'), rhs=res[:, :cw],
                                         start=True, stop=True)
                        hall = wpE.tile([128, 512], F16, tag="hall")
                        nc.scalar.activation(hall[:, :cw], ph[:, :cw], AF.Lrelu,
                                             bias=bias(f'b5_{r}'), alpha=0.01)
                        po = psEw.tile([2, 512], F32, tag="po")
                        nc.tensor.matmul(po[:, :cw], lhsT=wsl(wt, f'lin6_{r}'), rhs=hall[:, :cw],
                                         start=True, stop=True)
                        oo = wpE.tile([2, 512], F32, tag="oo")
                        nc.scalar.copy(oo[:, :cw], po[:, :cw])
                        nc.sync.dma_start(out_parts[r, :, c0:c0 + cw], oo[:, :cw])

            # software pipeline: hop0 x3 (each retriggers its AG for hop1),
            # then hop1 + epilogue per relation
            import os as _os
            _KPART = _os.environ.get("KPART", "ALL")
            with (tc.tile_pool(name="psG", bufs=2, space="PSUM") as psG,
                  tc.tile_pool(name="wpH", bufs=3) as wpH,
                  tc.tile_pool(name="psEt", bufs=2, space="PSUM") as psEt,
                  tc.tile_pool(name="psEw", bufs=1, space="PSUM") as psEw,
                  tc.tile_pool(name="wpE", bufs=2) as wpE):
                if _KPART != "A":
                    for r in range(R if _KPART in ("ALL", "H1") else 1):
                        process_hop(r, 0, psG, wpH)
                if _KPART in ("ALL", "H1"):
                    for r in range(R):
                        process_hop(r, 1, psG, wpH)
                        if _KPART == "ALL":
                            epilogue(r, psEt, psEw, wpE)

    nc.compile()


def kernel(**inp):
    meta, cores = _prep(inp)
    nc = bacc.Bacc("TRN2", target_bir_lowering=False, debug=False, num_devices=NCORES,
                   num_swdge_queues=NSWQ)
    _build(nc, meta)
    res = run_bass_kernel_spmd(nc, [dict(c) for c in cores], core_ids=list(range(NCORES)))
    out = np.zeros((N, C), np.float32)
    b6 = np.asarray(inp['b_lin6'], np.float32)
    for c in range(NCORES):
        parts = res.results[c]["out_parts"]
        out[c * NL:(c + 1) * NL] = parts.sum(axis=0).T[:NL] + b6[None, :]
    return out


if __name__ == "__main__":
    import reference
    inputs = {k: np.asarray(v) for k, v in reference.setup_inputs().items()}
    got = kernel(**inputs)
    exp = np.asarray(reference.reference(**inputs))
    err = np.abs(got - exp).max()
    rel = err / max(np.abs(exp).max(), 1e-9)
    print("abs err:", err, "rel err:", rel)


# revision 39
# speedup vs baseline: 2.1160x; 1.0917x over previous
"""BWGNN_Hetero Trainium2 kernel: 8-core SPMD, node-sharded graph/data parallel.

v2: fp16 tensor-engine path (merged M=128 LSTM gate matmuls, fp16 weights/
activations, fp16 feature upload), software-pipelined Phase B (all three
relations' hop-0 AllGathers issued at the end of Phase A; hops interleaved
across relations so collectives and tensor epilogues hide under the gpsimd
dma_gather stream), single fused attention epilogue pass, and one dma_gather
call per (tile-group, class).

Algorithmic reduction: the 5 beta-wavelet filters are linear combinations of
{f0, f1=L f0, f2=L f1} (L = normalized Laplacian), so each relation needs
only 2 sparse hops.
"""
import sys
sys.path.insert(0, '/opt/trn_rl_repo')
sys.path.insert(0, '/root/problem')

import numpy as np

import concourse.bacc as bacc
import concourse.bass as bass
import concourse.mybir as mybir
import concourse.tile as tile
from concourse.bass_utils import run_bass_kernel_spmd

F32 = mybir.dt.float32
F16 = mybir.dt.float16
I16 = mybir.dt.int16
A = mybir.AluOpType
AF = mybir.ActivationFunctionType

NCORES = 8
N, E, R, T = 50000, 800000, 3, 16
IV, IS, IP, H, C = 64, 64, 32, 128, 2
NL = N // NCORES            # 6250
NT = 49                     # dst tiles per core
NLP = NT * 128              # 6272 padded local nodes
NGP = NLP * NCORES          # 50176 padded global rows in AllGather output
LO_LIM = 32768              # int16 gather index limit
GS = 2                      # dst tiles per gather group
NSWQ = 4                    # SWDGE queues for gather descriptor rings
CHUNKS = [(i * 512, 512) for i in range(12)] + [(6144, 128)]

CTRUE = [[0.8, -0.5, 0.0],
         [3.0, -3.0, 0.75],
         [0.0, 3.0, -1.5],
         [0.0, 0.0, 0.75],
         [-0.2, 0.5, 0.0]]


def _wrap_idx(idx):
    """[n] int16 -> [128, ceil(n/16)] wrapped (i -> [i%16, i//16]) + replicated x8."""
    n = len(idx)
    L = max(1, (n + 15) // 16)
    a = np.zeros((16, L), np.int16)
    for p in range(16):
        vals = idx[p::16]
        a[p, :len(vals)] = vals
    return np.tile(a, (8, 1))


class WPack:
    def __init__(self, dtype):
        self.dtype = dtype
        self.cols = []
        self.off = 0
        self.slots = {}

    def add(self, name, mat, row0=0):
        mat = np.asarray(mat, self.dtype)
        k, m = mat.shape
        assert row0 + k <= 128
        buf = np.zeros((128, m), self.dtype)
        buf[row0:row0 + k] = mat
        self.cols.append(buf)
        self.slots[name] = (row0, k, self.off, m)
        self.off += m

    def image(self):
        return np.concatenate(self.cols, axis=1)


def _prep(inp):
    g = {k: np.asarray(v) for k, v in inp.items()}
    wp = WPack(np.float16)

    # torch gate-row offsets: i=0, f=64, g=128, o=192
    # XHv rows: [h_v (0:64); x_v (64:128)];  XHs rows: [x_s (0:64); h_s (64:128)]
    def lv_pair(g1, g2):
        m = np.zeros((128, 128), np.float32)
        for ci, ro in enumerate((g1, g2)):
            m[0:64, ci * 64:(ci + 1) * 64] = g['Whh_v'][ro:ro + 64, :].T
            m[64:128, ci * 64:(ci + 1) * 64] = g['Wih_v'][ro:ro + 64, :].T
        return m

    def ls_pair(g1, g2):
        m = np.zeros((128, 128), np.float32)
        for ci, ro in enumerate((g1, g2)):
            m[0:64, ci * 64:(ci + 1) * 64] = g['Wih_s'][ro:ro + 64, :].T
            m[64:128, ci * 64:(ci + 1) * 64] = g['Whh_s'][ro:ro + 64, :].T
        return m

    wp.add('lv_if', lv_pair(0, 64))
    wp.add('lv_og', lv_pair(192, 128))
    wp.add('ls_if', ls_pair(0, 64))
    wp.add('ls_og', ls_pair(192, 128))
    wp.add('lin', g['W_lin'].T)                       # rows 0:64 (rhs = h_v at base 0)
    wp.add('lin1', g['W_lin1'].T, row0=64)            # rows 64:128 (rhs = h_s at base 64)
    wp.add('pers', g['W_pers'].T)
    wp.add('lin2a', g['W_lin2'][:, 0:128].T)
    wp.add('lin2b', g['W_lin2'][:, 128:256].T)
    wp.add('lin3a', g['W_lin3'][:, 0:128].T)
    wp.add('lin3b', g['W_lin3'][:, 128:256].T)
    wp.add('lin4a', g['W_lin4'][:, 0:128].T)
    wp.add('lin4bd', (g['W_lin4'][:, 128:256] + g['W_lin4'][:, 384:512]).T)
    wp.add('lin4c', g['W_lin4'][:, 256:384].T)
    for r in range(R):
        for o in range(5):
            for j in range(3):
                if CTRUE[o][j] != 0.0:
                    wp.add(f'wf1_{r}_{o}_{j}', (CTRUE[o][j] * g['Wf1'][r]).T)
        wp.add(f'wf2_{r}', g['Wf2'][r][:, None])
        wp.add(f'lin5_{r}', g['W_lin5'][r].T)
    for k in range(6):
        wp.add(f'lin6_{k}', g['W_lin6'][:, k * 128:(k + 1) * 128].T)
    wp.add('ident', np.eye(128, dtype=np.float32))
    wp.add('one11', np.ones((1, 1), np.float32))
    for o in range(5):
        for j in range(3):
            wp.add(f'c_{o}_{j}', np.array([[CTRUE[o][j]]], np.float32))
    wimg = wp.image()

    bcols, blist = {}, []

    def addb(name, vec):
        bcols[name] = len(blist)
        v = np.zeros((128, 1), np.float32)
        v[:len(vec), 0] = np.asarray(vec, np.float32).ravel()
        blist.append(v)

    bv = g['bih_v'] + g['bhh_v']
    bs = g['bih_s'] + g['bhh_s']
    addb('b_if_v', np.concatenate([bv[0:64], bv[64:128]]))     # [i_v; f_v]
    addb('b_o_v', bv[192:256])
    addb('b_g_v', bv[128:192])
    addb('b_i_s', bs[0:64])
    addb('b_f_s', bs[64:128])
    addb('b_o_s', bs[192:256])
    addb('b_g_s', bs[128:192])
    addb('b_lin', g['b_lin'])
    addb('b_lin1', g['b_lin1'])
    addb('b_pers', g['b_pers'])
    addb('b2', g['b_lin2'])
    addb('b3', g['b_lin3'])
    addb('b4', g['b_lin4'])
    for r in range(R):
        addb(f'bf1_{r}', g['bf1'][r])
        addb(f'b5_{r}', g['b_lin5'][r])
    bimg = np.concatenate(blist, axis=1)

    src = np.asarray(g['src'], np.int64)
    dst = np.asarray(g['dst'], np.int64)
    gsrc_all = (src // NL) * NLP + (src % NL)

    percore = [dict() for _ in range(NCORES)]
    relmeta = []
    for r in range(R):
        deg = np.bincount(dst[r], minlength=N).astype(np.float32)
        dinv = np.clip(deg, 1.0, None) ** -0.5

        # bucket edges: (core, tile, class)
        per = []
        for c in range(NCORES):
            m = (dst[r] // NL) == c
            sc = gsrc_all[r][m]
            dl = dst[r][m] - c * NL
            tl, col = dl // 128, dl % 128
            tiles = []
            for t in range(NT):
                mt = tl == t
                st_, ct_ = sc[mt], col[mt]
                lo = st_ < LO_LIM
                tiles.append((st_[lo], ct_[lo], st_[~lo] - LO_LIM, ct_[~lo]))
            per.append(tiles)
        # common (max-over-cores) window counts
        lo_w = [max(1, max((len(per[c][t][0]) + 127) // 128 for c in range(NCORES)))
                for t in range(NT)]
        hi_w = [max(1, max((len(per[c][t][2]) + 127) // 128 for c in range(NCORES)))
                for t in range(NT)]
        relmeta.append({'lo_w': lo_w, 'hi_w': hi_w})
        for c in range(NCORES):
            li_s, lc_s, hi_s, hc_s = [], [], [], []
            for t in range(NT):
                li, lc, hi, hc = per[c][t]
                lp = np.zeros(lo_w[t] * 128, np.int64); lp[:len(li)] = li
                lcp = np.full(lo_w[t] * 128, -1, np.int64); lcp[:len(lc)] = lc
                hp = np.zeros(hi_w[t] * 128, np.int64); hp[:len(hi)] = hi
                hcp = np.full(hi_w[t] * 128, -1, np.int64); hcp[:len(hc)] = hc
                li_s.append(lp); lc_s.append(lcp); hi_s.append(hp); hc_s.append(hcp)
            li_s = np.concatenate(li_s); lc_s = np.concatenate(lc_s)
            hi_s = np.concatenate(hi_s); hc_s = np.concatenate(hc_s)

            def mkind(colarr):
                # wrapped layout [e, w*128 + c]: partition = edge-in-window, so the
                # device DMA is a contiguous per-partition copy (no 256B scatter)
                W = len(colarr) // 128
                ind = np.zeros((128, W * 128), np.float16)
                valid = colarr >= 0
                pos = np.nonzero(valid)[0]
                w, e = pos // 128, pos % 128
                ind[e, w * 128 + colarr[valid]] = 1.0
                return ind
            pc = percore[c]
            pc[f'gidx_lo_{r}'] = _wrap_idx(li_s.astype(np.int16))
            pc[f'gidx_hi_{r}'] = _wrap_idx(hi_s.astype(np.int16))
            pc[f'ind_lo_{r}'] = mkind(lc_s)
            pc[f'ind_hi_{r}'] = mkind(hc_s)
            dp = pc.setdefault('_dinv', np.zeros((128, 2 * R * NT), np.float32))
            dvl = np.ones(NLP, np.float32)
            dvl[:NL] = dinv[c * NL:(c + 1) * NL]
            dp[:, r * NT:(r + 1) * NT] = dvl.reshape(NT, 128).T
            dp[:, R * NT + r * NT:R * NT + (r + 1) * NT] = -dvl.reshape(NT, 128).T

    voc = np.asarray(g['voc_features'], np.float16)
    sms = np.asarray(g['sms_features'], np.float16)
    pers = np.asarray(g['personal_feature'], np.float16)
    cores = []
    for c in range(NCORES):
        pc = percore[c]
        sl = slice(c * NL, (c + 1) * NL)
        vt = np.zeros((T, IV, NLP), np.float16)
        st_ = np.zeros((T, IS, NLP), np.float16)
        vt[:, :, :NL] = voc[sl].transpose(1, 2, 0)
        st_[:, :, :NL] = sms[sl].transpose(1, 2, 0)
        pt = np.zeros((IP, NLP), np.float16)
        pt[:, :NL] = pers[sl].T
        pc['voc'] = vt
        pc['sms'] = st_
        pc['pers'] = pt
        pc['wpack'] = wimg
        pc['bpack'] = bimg
        pc['dpack'] = pc.pop('_dinv')
        cores.append(pc)
    meta = {
        'wp': wp.slots, 'bcols': bcols, 'rel': relmeta,
        'shapes': {k: v.shape for k, v in cores[0].items()},
        'dtypes': {k: v.dtype for k, v in cores[0].items()},
    }
    return meta, cores


def _build(nc, meta):
    sh, dt = meta['shapes'], meta['dtypes']
    WP, BC = meta['wp'], meta['bcols']
    inputs = {k: nc.dram_tensor(k, list(sh[k]), mybir.dt.from_np(np.dtype(dt[k])),
                                kind="ExternalInput") for k in sh}
    out_parts = nc.dram_tensor("out_parts", [4, 2, NLP], F32, kind="ExternalOutput")

    def wsl(wt, name):
        r0, k, off, m = WP[name]
        return wt[r0:r0 + k, off:off + m]

    with tile.TileContext(nc) as tc:
        with (
            tc.tile_pool(name="const", bufs=1) as cpool,
            tc.tile_pool(name="persist", bufs=1) as spool,
            tc.tile_pool(name="dram", bufs=1, space="DRAM") as dpool,
        ):
            wt = cpool.tile([128, sh['wpack'][1]], F16)
            nc.sync.dma_start(wt[:], inputs['wpack'][:])
            bt = cpool.tile([128, sh['bpack'][1]], F32)
            nc.sync.dma_start(bt[:], inputs['bpack'][:])
            dpt = cpool.tile([128, 2 * R * NT], F32)
            nc.sync.dma_start(dpt[:], inputs['dpack'][:])
            onesf16 = cpool.tile([1, 128], F16)
            nc.vector.memset(onesf16[:], 1.0)

            def bias(name, p0=0, pn=128):
                return bt[p0:p0 + pn, BC[name]:BC[name] + 1]

            # persistent fp16 node-major filter states: FA_r (f0, later f2), FB_r (f1)
            FA = [spool.tile([128, NLP], F16, name=f"FA{r}") for r in range(R)]
            FB = [spool.tile([128, NLP], F16, name=f"FB{r}") for r in range(R)]
            xin_dram = [dpool.tile([128, NLP], F16, tag=f"xin{r}", name=f"xind{r}")
                        for r in range(R)]
            ml = [[dpool.tile([NLP, H], F16, tag=f"ml{r}h{h}", name=f"mld{r}h{h}")
                   for h in range(2)] for r in range(R)]
            mf = [[dpool.tile([NGP, H], F16, tag=f"mf{r}h{h}", name=f"mfd{r}h{h}",
                              addr_space="Shared") for h in range(2)] for r in range(R)]

            def dv(r, t):
                return dpt[:, r * NT + t:r * NT + t + 1]

            def ndv(r, t):
                return dpt[:, R * NT + r * NT + t:R * NT + r * NT + t + 1]

            # =============== Phase A: fused LSTMs ===============
            with (tc.tile_pool(name="stA", bufs=1) as stA,
                  tc.tile_pool(name="wpA", bufs=2) as wpA):
                XHv = stA.tile([128, NLP], F16)
                XHs = stA.tile([128, NLP], F16)
                Cst = stA.tile([128, NLP], F32)   # rows 64:128 = c_v, rows 0:64 = c_s
                nc.vector.memset(XHv[0:64, :], 0.0)
                nc.vector.memset(XHs[64:128, :], 0.0)
                nc.vector.memset(Cst[:], 0.0)
                with tc.tile_pool(name="psA", bufs=2, space="PSUM") as psA:
                    for t in range(T):
                        nc.sync.dma_start(XHv[64:128, :], inputs['voc'][t])
                        nc.sync.dma_start(XHs[0:64, :], inputs['sms'][t])
                        for (c0, cw) in CHUNKS:
                            PA = psA.tile([128, 512], F32, tag="pa")  # [i_v; f_v]
                            PB = psA.tile([128, 512], F32, tag="pb")  # [o_v; g_v]
                            PC = psA.tile([128, 512], F32, tag="pc")  # [i_s; f_s]
                            PD = psA.tile([128, 512], F32, tag="pd")  # [o_s; g_s]
                            nc.tensor.matmul(PA[:, :cw], lhsT=wsl(wt, 'lv_if'),
                                             rhs=XHv[:, c0:c0 + cw], start=True, stop=True)
                            nc.tensor.matmul(PB[:, :cw], lhsT=wsl(wt, 'lv_og'),
                                             rhs=XHv[:, c0:c0 + cw], start=True, stop=True)
                            nc.tensor.matmul(PC[:, :cw], lhsT=wsl(wt, 'ls_if'),
                                             rhs=XHs[:, c0:c0 + cw], start=True, stop=True)
                            nc.tensor.matmul(PD[:, :cw], lhsT=wsl(wt, 'ls_og'),
                                             rhs=XHs[:, c0:c0 + cw], start=True, stop=True)
                            # v-source: SA = sigmoid([i_v; f_v]) full 128
                            SA = wpA.tile([128, 512], F16, tag="SA")
                            nc.scalar.activation(SA[:, :cw], PA[:, :cw], AF.Sigmoid,
                                                 bias=bias('b_if_v'))
                            Sov = wpA.tile([64, 512], F16, tag="Sov")
                            nc.scalar.activation(Sov[:, :cw], PB[0:64, :cw], AF.Sigmoid,
                                                 bias=bias('b_o_v', 0, 64))
                            Tgv = wpA.tile([64, 512], F16, tag="Tgv")
                            nc.scalar.activation(Tgv[:, :cw], PB[64:128, :cw], AF.Tanh,
                                                 bias=bias('b_g_v', 0, 64))
                            # s-source split sigmoids (base-0 outputs)
                            Sis = wpA.tile([64, 512], F16, tag="Sis")
                            nc.scalar.activation(Sis[:, :cw], PC[0:64, :cw], AF.Sigmoid,
                                                 bias=bias('b_i_s', 0, 64))
                            Sfs = wpA.tile([64, 512], F16, tag="Sfs")
                            nc.scalar.activation(Sfs[:, :cw], PC[64:128, :cw], AF.Sigmoid,
                                                 bias=bias('b_f_s', 0, 64))
                            Sos = wpA.tile([64, 512], F16, tag="Sos")
                            nc.scalar.activation(Sos[:, :cw], PD[0:64, :cw], AF.Sigmoid,
                                                 bias=bias('b_o_s', 0, 64))
                            Tgs = wpA.tile([64, 512], F16, tag="Tgs")
                            nc.scalar.activation(Tgs[:, :cw], PD[64:128, :cw], AF.Tanh,
                                                 bias=bias('b_g_s', 0, 64))
                            # c_v (rows 64:128 of Cst): u = f_v*c_v ; v = i_v*g_v
                            uv = wpA.tile([64, 512], F32, tag="uv")
                            nc.vector.scalar_tensor_tensor(uv[:, :cw], SA[64:128, :cw], 0.0,
                                                           Cst[64:128, c0:c0 + cw],
                                                           op0=A.bypass, op1=A.mult)
                            vv = wpA.tile([64, 512], F32, tag="vv")
                            nc.vector.scalar_tensor_tensor(vv[:, :cw], SA[0:64, :cw], 0.0,
                                                           Tgv[:, :cw], op0=A.bypass, op1=A.mult)
                            nc.vector.scalar_tensor_tensor(Cst[64:128, c0:c0 + cw], uv[:, :cw],
                                                           0.0, vv[:, :cw],
                                                           op0=A.bypass, op1=A.add)
                            tcv = wpA.tile([64, 512], F32, tag="tcv")
                            nc.scalar.activation(tcv[:, :cw], Cst[64:128, c0:c0 + cw], AF.Tanh)
                            nc.vector.scalar_tensor_tensor(XHv[0:64, c0:c0 + cw], Sov[:, :cw],
                                                           0.0, tcv[:, :cw],
                                                           op0=A.bypass, op1=A.mult)
                            # c_s (rows 0:64 of Cst)
                            us = wpA.tile([64, 512], F32, tag="us")
                            nc.vector.scalar_tensor_tensor(us[:, :cw], Sfs[:, :cw], 0.0,
                                                           Cst[0:64, c0:c0 + cw],
                                                           op0=A.bypass, op1=A.mult)
                            vs = wpA.tile([64, 512], F32, tag="vs")
                            nc.vector.scalar_tensor_tensor(vs[:, :cw], Sis[:, :cw], 0.0,
                                                           Tgs[:, :cw], op0=A.bypass, op1=A.mult)
                            nc.vector.scalar_tensor_tensor(Cst[0:64, c0:c0 + cw], us[:, :cw],
                                                           0.0, vs[:, :cw],
                                                           op0=A.bypass, op1=A.add)
                            tcs = wpA.tile([64, 512], F32, tag="tcs")
                            nc.scalar.activation(tcs[:, :cw], Cst[0:64, c0:c0 + cw], AF.Tanh)
                            nc.vector.scalar_tensor_tensor(XHs[64:128, c0:c0 + cw], Sos[:, :cw],
                                                           0.0, tcs[:, :cw],
                                                           op0=A.bypass, op1=A.mult)

                # ---- Phase A epilogue: head + inline transposes/messages
                PT = stA.tile([32, NLP], F16)
                nc.sync.dma_start(PT[:], inputs['pers'][:])
                with (tc.tile_pool(name="psB", bufs=2, space="PSUM") as psB,
                      tc.tile_pool(name="psBs", bufs=2, space="PSUM") as psBs):
                    for (c0, cw) in CHUNKS:
                        pxa = psB.tile([128, 512], F32, tag="pa")
                        nc.tensor.matmul(pxa[:, :cw], lhsT=wsl(wt, 'lin'),
                                         rhs=XHv[0:64, c0:c0 + cw], start=True, stop=True)
                        pxp = psB.tile([128, 512], F32, tag="pb")
                        nc.tensor.matmul(pxp[:, :cw], lhsT=wsl(wt, 'pers'),
                                         rhs=PT[:, c0:c0 + cw], start=True, stop=True)
                        pxs = psB.tile([128, 512], F32, tag="pc")
                        nc.tensor.matmul(pxs[:, :cw], lhsT=wsl(wt, 'lin1'),
                                         rhs=XHs[64:128, c0:c0 + cw], start=True, stop=True)
                        XA = wpA.tile([128, 512], F16, tag="XA")
                        XP = wpA.tile([128, 512], F16, tag="XP")
                        XS = wpA.tile([128, 512], F16, tag="XS")
                        nc.scalar.activation(XA[:, :cw], pxa[:, :cw], AF.Lrelu, bias=bias('b_lin'), alpha=0.01)
                        nc.scalar.activation(XP[:, :cw], pxp[:, :cw], AF.Lrelu, bias=bias('b_pers'), alpha=0.01)
                        nc.scalar.activation(XS[:, :cw], pxs[:, :cw], AF.Lrelu, bias=bias('b_lin1'), alpha=0.01)
                        p0 = psB.tile([128, 512], F32, tag="pa")
                        nc.tensor.matmul(p0[:, :cw], lhsT=wsl(wt, 'lin2a'), rhs=XA[:, :cw], start=True, stop=False)
                        nc.tensor.matmul(p0[:, :cw], lhsT=wsl(wt, 'lin2b'), rhs=XP[:, :cw], start=False, stop=True)
                        p1 = psB.tile([128, 512], F32, tag="pb")
                        nc.tensor.matmul(p1[:, :cw], lhsT=wsl(wt, 'lin3a'), rhs=XS[:, :cw], start=True, stop=False)
                        nc.tensor.matmul(p1[:, :cw], lhsT=wsl(wt, 'lin3b'), rhs=XP[:, :cw], start=False, stop=True)
                        p2 = psB.tile([128, 512], F32, tag="pc")
                        nc.tensor.matmul(p2[:, :cw], lhsT=wsl(wt, 'lin4a'), rhs=XA[:, :cw], start=True, stop=False)
                        nc.tensor.matmul(p2[:, :cw], lhsT=wsl(wt, 'lin4bd'), rhs=XP[:, :cw], start=False, stop=False)
                        nc.tensor.matmul(p2[:, :cw], lhsT=wsl(wt, 'lin4c'), rhs=XS[:, :cw], start=False, stop=True)
                        Xc = [wpA.tile([128, 512], F16, tag=f"X{r}c", name=f"Xc{r}") for r in range(R)]
                        nc.scalar.activation(Xc[0][:, :cw], p0[:, :cw], AF.Lrelu, bias=bias('b2'), alpha=0.01)
                        nc.scalar.activation(Xc[1][:, :cw], p1[:, :cw], AF.Lrelu, bias=bias('b3'), alpha=0.01)
                        nc.scalar.activation(Xc[2][:, :cw], p2[:, :cw], AF.Lrelu, bias=bias('b4'), alpha=0.01)
                        for r in range(R):
                            nc.sync.dma_start(xin_dram[r][:, c0:c0 + cw], Xc[r][:, :cw])
                        p6 = psBs.tile([2, 512], F32, tag="p6")
                        nc.tensor.matmul(p6[:, :cw], lhsT=wsl(wt, 'lin6_3'), rhs=Xc[0][:, :cw],
                                         start=True, stop=False)
                        nc.tensor.matmul(p6[:, :cw], lhsT=wsl(wt, 'lin6_4'), rhs=Xc[1][:, :cw],
                                         start=False, stop=False)
                        nc.tensor.matmul(p6[:, :cw], lhsT=wsl(wt, 'lin6_5'), rhs=Xc[2][:, :cw],
                                         start=False, stop=True)
                        o6 = wpA.tile([2, 512], F32, tag="o6")
                        nc.scalar.copy(o6[:, :cw], p6[:, :cw])
                        nc.sync.dma_start(out_parts[3, :, c0:c0 + cw], o6[:, :cw])

            # transposes -> F0 (node-major) + hop-0 messages + hop-0 AllGathers
            with (tc.tile_pool(name="psT", bufs=2, space="PSUM") as psT,
                  tc.tile_pool(name="wpT", bufs=2) as wpT):
                for r in range(R):
                    for t in range(NT):
                        xt = wpT.tile([128, 128], F16, tag="xt")
                        nc.sync.dma_start(xt[:], xin_dram[r][:, t * 128:(t + 1) * 128])
                        tr = psT.tile([128, 128], F16, tag="tr")
                        nc.tensor.transpose(tr[:], xt[:], wsl(wt, 'ident'))
                        nc.vector.tensor_copy(FA[r][:, t * 128:(t + 1) * 128], tr[:])
                        m1 = wpT.tile([128, 128], F16, tag="m1")
                        nc.vector.tensor_scalar_mul(m1[:], tr[:], dv(r, t))
                        nc.sync.dma_start(ml[r][0][t * 128:(t + 1) * 128, :], m1[:])
                    nc.gpsimd.collective_compute(
                        "AllGather", A.bypass,
                        replica_groups=[list(range(NCORES))],
                        ins=[ml[r][0].opt()], outs=[mf[r][0].opt()],
                    )

            # =============== Phase B ===============
            MAXLO = max(max(sum(meta['rel'][r]['lo_w'][t0:t0 + GS])
                            for t0 in range(0, NT, GS)) for r in range(R))
            MAXHI = max(max(sum(meta['rel'][r]['hi_w'][t0:t0 + GS])
                            for t0 in range(0, NT, GS)) for r in range(R))

            gq = [0]            # round-robin gather queue selector

            def process_hop(r, hop, psG, wpH):
                relm = meta['rel'][r]
                lo_w, hi_w = relm['lo_w'], relm['hi_w']
                lo_off, hi_off = [0], [0]
                for t in range(NT):
                    lo_off.append(lo_off[-1] + lo_w[t])
                    hi_off.append(hi_off[-1] + hi_w[t])
                fsrc = FA[r] if hop == 0 else FB[r]
                fdst = FB[r] if hop == 0 else FA[r]
                if True:
                    for t0g in range(0, NT, GS):
                        tiles = list(range(t0g, min(t0g + GS, NT)))
                        nlo = sum(lo_w[t] for t in tiles)
                        nhi = sum(hi_w[t] for t in tiles)
                        bufs = {}
                        for cls, nwin, mx, woff in (('lo', nlo, MAXLO, lo_off[tiles[0]]),
                                                    ('hi', nhi, MAXHI, hi_off[tiles[0]])):
                            it = wpH.tile([128, mx * 8], I16, tag=f"idx{cls}")
                            nc.sync.dma_start(it[:, :nwin * 8],
                                              inputs[f'gidx_{cls}_{r}'][:, woff * 8:(woff + nwin) * 8])
                            gb = wpH.tile([128, mx, 128], F16, tag=f"gb{cls}")
                            mfh = mf[r][hop]
                            in_ap = mfh[0:LO_LIM, :] if cls == 'lo' else mfh[LO_LIM:NGP, :]
                            GW = 8
                            for w0 in range(0, nwin, GW):
                                sw = min(GW, nwin - w0)
                                q = gq[0]
                                gq[0] = (q + 1) % NSWQ
                                nc.gpsimd.dma_gather(
                                    out_ap=gb[:, w0:w0 + sw, :], in_ap=in_ap,
                                    idxs_ap=it[:, w0 * 8:(w0 + sw) * 8],
                                    num_idxs=sw * 128, num_idxs_reg=sw * 128,
                                    elem_size=H, queue_num=q)
                            ib = wpH.tile([128, mx, 128], F16, tag=f"ib{cls}")
                            nc.sync.dma_start(
                                ib[:, :nwin, :],
                                inputs[f'ind_{cls}_{r}'].ap()[:, woff * 128:(woff + nwin) * 128]
                                .rearrange("e (w d) -> e w d", w=nwin))
                            bufs[cls] = (gb, ib)
                        for t in tiles:
                            agg = psG.tile([128, 128], F32, tag="agg")
                            wins = ([('lo', lo_off[t] - lo_off[tiles[0]] + w) for w in range(lo_w[t])]
                                    + [('hi', hi_off[t] - hi_off[tiles[0]] + w) for w in range(hi_w[t])])
                            for wi, (cls, w) in enumerate(wins):
                                gb, ib = bufs[cls]
                                nc.tensor.matmul(agg[:], lhsT=ib[:, w, :], rhs=gb[:, w, :],
                                                 start=(wi == 0), stop=(wi == len(wins) - 1))
                            nc.vector.scalar_tensor_tensor(
                                fdst[:, t * 128:(t + 1) * 128], agg[:], ndv(r, t),
                                fsrc[:, t * 128:(t + 1) * 128],
                                op0=A.mult, op1=A.add)
                            if hop == 0:
                                m1 = wpH.tile([128, 128], F16, tag="m1")
                                # scalar engine is idle in Phase B; DVE tensor_scalar
                                # stalls ~4us here (SWDGE ring contention)
                                nc.scalar.activation(
                                    m1[:], fdst[:, t * 128:(t + 1) * 128], AF.Copy,
                                    scale=dv(r, t))
                                nc.sync.dma_start(ml[r][1][t * 128:(t + 1) * 128, :], m1[:])
                if hop == 0:
                    nc.gpsimd.collective_compute(
                        "AllGather", A.bypass,
                        replica_groups=[list(range(NCORES))],
                        ins=[ml[r][1].opt()], outs=[mf[r][1].opt()],
                    )

            def epilogue(r, psEt, psEw, wpE):
                # F1 = FB[r], F2 = FA[r] (node-major fp16); x_in reloaded from DRAM
                if True:
                    for (c0, cw) in CHUNKS:
                        nsub = cw // 128
                        X0c = wpE.tile([128, 512], F16, tag="X0c")
                        nc.sync.dma_start(X0c[:, :cw], xin_dram[r][:, c0:c0 + cw])
                        F1c = wpE.tile([128, 512], F16, tag="F1c")
                        F2c = wpE.tile([128, 512], F16, tag="F2c")
                        for si in range(nsub):
                            tr = psEt.tile([128, 128], F16, tag="tr")
                            nc.tensor.transpose(tr[:], FB[r][:, c0 + si * 128:c0 + (si + 1) * 128],
                                                wsl(wt, 'ident'))
                            nc.vector.tensor_copy(F1c[:, si * 128:(si + 1) * 128], tr[:])
                            tr2 = psEt.tile([128, 128], F16, tag="tr")
                            nc.tensor.transpose(tr2[:], FA[r][:, c0 + si * 128:c0 + (si + 1) * 128],
                                                wsl(wt, 'ident'))
                            nc.vector.tensor_copy(F2c[:, si * 128:(si + 1) * 128], tr2[:])
                        den_sb = wpE.tile([1, 512], F32, tag="den")
                        gsb = [wpE.tile([1, 512], F32, tag=f"gs{j}", name=f"gsb{j}")
                               for j in range(3)]
                        Bsrc = [X0c[:, :cw], F1c[:, :cw], F2c[:, :cw]]
                        for o in range(5):
                            pso = psEw.tile([128, 512], F32, tag="to")
                            js = [j for j in range(3) if CTRUE[o][j] != 0.0]
                            for ji, j in enumerate(js):
                                nc.tensor.matmul(pso[:, :cw], lhsT=wsl(wt, f'wf1_{r}_{o}_{j}'),
                                                 rhs=Bsrc[j], start=(ji == 0), stop=(ji == len(js) - 1))
                            To = wpE.tile([128, 512], F16, tag="To")
                            nc.scalar.activation(To[:, :cw], pso[:, :cw], AF.Tanh, bias=bias(f'bf1_{r}'))
                            psc = psEw.tile([1, 512], F32, tag="sc")
                            nc.tensor.matmul(psc[:, :cw], lhsT=wsl(wt, f'wf2_{r}'), rhs=To[:, :cw],
                                             start=True, stop=True)
                            eo = wpE.tile([1, 512], F16, tag="eo")
                            nc.scalar.activation(eo[:, :cw], psc[:, :cw], AF.Exp)
                            if o == 0:
                                nc.vector.tensor_copy(den_sb[:, :cw], eo[:, :cw])
                                for j in range(3):
                                    nc.vector.tensor_scalar_mul(gsb[j][:, :cw], eo[:, :cw],
                                                                float(CTRUE[o][j]))
                            else:
                                nc.vector.scalar_tensor_tensor(den_sb[:, :cw], eo[:, :cw], 0.0,
                                                               den_sb[:, :cw],
                                                               op0=A.bypass, op1=A.add)
                                for j in range(3):
                                    if CTRUE[o][j] != 0.0:
                                        nc.vector.scalar_tensor_tensor(
                                            gsb[j][:, :cw], eo[:, :cw], float(CTRUE[o][j]),
                                            gsb[j][:, :cw], op0=A.mult, op1=A.add)
                        rec = wpE.tile([1, 512], F32, tag="rec")
                        nc.vector.reciprocal(rec[:, :cw], den_sb[:, :cw])
                        res = wpE.tile([128, 512], F16, tag="res")
                        tmp = wpE.tile([128, 512], F16, tag="tmp")
                        for j in range(3):
                            gj = wpE.tile([1, 512], F16, tag="gj")
                            nc.vector.scalar_tensor_tensor(gj[:, :cw], rec[:, :cw], 0.0,
                                                           gsb[j][:, :cw], op0=A.bypass, op1=A.mult)
                            pbj = psEw.tile([128, 512], F32, tag="bc")
                            nc.tensor.matmul(pbj[:, :cw], lhsT=onesf16[:], rhs=gj[:, :cw],
                                             start=True, stop=True)
                            if j == 0:
                                nc.vector.scalar_tensor_tensor(res[:, :cw], Bsrc[j], 0.0, pbj[:, :cw],
                                                               op0=A.bypass, op1=A.mult)
                            else:
                                nc.vector.scalar_tensor_tensor(tmp[:, :cw], Bsrc[j], 0.0, pbj[:, :cw],
                                                               op0=A.bypass, op1=A.mult)
                                nc.vector.scalar_tensor_tensor(res[:, :cw], res[:, :cw], 0.0,
                                                               tmp[:, :cw], op0=A.bypass, op1=A.add)
                        ph = psEw.tile([128, 512], F32, tag="to")
                        nc.tensor.matmul(ph[:, :cw], lhsT=wsl(wt, f'lin5_{r}'), rhs=res[:, :cw],
                                         start=True, stop=True)
                        hall = wpE.tile([128, 512], F16, tag="hall")
                        nc.scalar.activation(hall[:, :cw], ph[:, :cw], AF.Lrelu,
                                             bias=bias(f'b5_{r}'), alpha=0.01)
                        po = psEw.tile([2, 512], F32, tag="po")
                        nc.tensor.matmul(po[:, :cw], lhsT=wsl(wt, f'lin6_{r}'), rhs=hall[:, :cw],
                                         start=True, stop=True)
                        oo = wpE.tile([2, 512], F32, tag="oo")
                        nc.scalar.copy(oo[:, :cw], po[:, :cw])
                        nc.sync.dma_start(out_parts[r, :, c0:c0 + cw], oo[:, :cw])

            # software pipeline: hop0 x3 (each retriggers its AG for hop1),
            # then hop1 + epilogue per relation
            import os as _os
            _KPART = _os.environ.get("KPART", "ALL")
            with (tc.tile_pool(name="psG", bufs=2, space="PSUM") as psG,
                  tc.tile_pool(name="wpH", bufs=3) as wpH,
                  tc.tile_pool(name="psEt", bufs=2, space="PSUM") as psEt,
                  tc.tile_pool(name="psEw", bufs=1, space="PSUM") as psEw,
                  tc.tile_pool(name="wpE", bufs=2) as wpE):
                if _KPART != "A":
                    for r in range(R if _KPART in ("ALL", "H1") else 1):
                        process_hop(r, 0, psG, wpH)
                if _KPART in ("ALL", "H1"):
                    for r in range(R):
                        process_hop(r, 1, psG, wpH)
                        if _KPART == "ALL":
                            epilogue(r, psEt, psEw, wpE)

    nc.compile()


def kernel(**inp):
    meta, cores = _prep(inp)
    nc = bacc.Bacc("TRN2", target_bir_lowering=False, debug=False, num_devices=NCORES,
                   num_swdge_queues=NSWQ)
    _build(nc, meta)
    res = run_bass_kernel_spmd(nc, [dict(c) for c in cores], core_ids=list(range(NCORES)))
    out = np.zeros((N, C), np.float32)
    b6 = np.asarray(inp['b_lin6'], np.float32)
    for c in range(NCORES):
        parts = res.results[c]["out_parts"]
        out[c * NL:(c + 1) * NL] = parts.sum(axis=0).T[:NL] + b6[None, :]
    return out


if __name__ == "__main__":
    import reference
    inputs = {k: np.asarray(v) for k, v in reference.setup_inputs().items()}
    got = kernel(**inputs)
    exp = np.asarray(reference.reference(**inputs))
    err = np.abs(got - exp).max()
    rel = err / max(np.abs(exp).max(), 1e-9)
    print("abs err:", err, "rel err:", rel)
